# revision 6
# baseline (speedup 1.0000x reference)
"""Trainium2 Bass kernel for the AutoRegressiveLSTM problem.

Strategy: data-parallel over batch (512 -> 64 rows per NeuronCore, 8 cores,
zero inter-core communication). All weights resident in SBUF as bf16;
matmuls in bf16 with fp32 PSUM accumulation (validated ~4e-3 max rel err
through the full 95-step recurrence).

Per-core layouts:
  - LSTM state h is kept TRANSPOSED (hT, [unit, batch]) because the
    TensorEngine computes out = lhsT.T @ rhs: z[batch, gates] needs
    stationary hT k-tiles [128 units, 64 batch].
  - Gate pre-activations z land in PSUM "gate-folded": each [128, 512]
    PSUM tile holds one gate, partitions 0:64 = units 0:512 (batch-major),
    partitions 64:128 = units 512:1024. The two halves are two independent
    matmul accumulation chains targeting different PE column groups, which
    the hardware runs concurrently (recovers full 128-wide array
    utilization despite the 64-row batch shard).
  - c / h state stays in the same folded [128, 512] layout, so all
    elementwise ops run at full 128-partition width.
  - h is un-folded back to hT via 8 PE transposes per cell.
  - pred (the Dense output) is computed transposed (predT = Wd.T @ hT) and
    written per-step to DRAM as [128 feat, 64 batch]; the host unshards.

The double normalization of the input and W2+U2 (cell2 sees x == h) are
folded on the host.
"""

from contextlib import ExitStack

import numpy as np
import ml_dtypes

import concourse.bass as bass
import concourse.tile as tile
from concourse import bacc, mybir
from concourse.bass_utils import run_bass_kernel_spmd

BF16 = mybir.dt.bfloat16
F32 = mybir.dt.float32
AF = mybir.ActivationFunctionType

NCORES = 8
B_FULL = 512
BS = B_FULL // NCORES   # 64 batch rows per core
T = 64                  # warmup sequence length
F = 128                 # features
U = 1024                # LSTM units
G = 4 * U               # 4096 gate columns
NK = U // 128           # 8 contraction k-tiles
OUT_STEPS = 32
EPS = 1e-7

# gate column ranges in the natural [i f g o] weight layout, split in two
# 512-wide halves; each pair (lo, hi) is one gate's two halves and maps to
# PSUM partitions [0:64] / [64:128] of one [128, 512] tile.
GATE_OFF = {"i": 0, "f": U, "g": 2 * U, "o": 3 * U}
H = 512  # half-gate width

_BUILD_CACHE = {}


def _emit_cell(nc, pools, stats, rhs_of, c_fold, first, bias_tile=None):
    """One LSTM cell: z = sum_k stats[k].T @ rhs_of(k, col) for all four
    gates, activations, state update. Returns h_fold [128, 512] bf16.

    stats: list of stationary APs ([128, <=128] bf16 k-tiles)
    rhs_of: (k, col_off) -> moving AP [128, 512]
    c_fold: persistent cell-state tile [128, 512] f32
    first: if True, h==0 and c==0 on entry (skip f*c, direct init)
    """
    psum, gates, temps = pools["psum"], pools["gates"], pools["temps"]
    nk = len(stats)
    # order: g, i, f, o  (c needs g,i,f; o only feeds the output tail)
    zp = {}
    for gate in ("g", "i", "f", "o"):
        off = GATE_OFF[gate]
        z = psum.tile([128, H], F32, tag="zp")
        for k in range(nk):
            # the two chains write disjoint partition halves of one bank;
            # the sim's zero-region group check is partition-blind, so skip
            nc.tensor.matmul(
                z[0:64, :], stats[k], rhs_of(k, off),
                start=(k == 0), stop=(k == nk - 1), skip_group_check=True,
            )
            nc.tensor.matmul(
                z[64:128, :], stats[k], rhs_of(k, off + H),
                start=(k == 0), stop=(k == nk - 1), skip_group_check=True,
            )
        if bias_tile is not None:
            nc.vector.tensor_add(z[:, :], z[:, :], bias_tile[gate])
        zp[gate] = z

        # activation as soon as the gate's matmuls are done
        if gate == "g":
            g_t = gates.tile([128, H], F32, tag="gact")
            nc.scalar.activation(g_t, z[:, :], AF.Tanh)
        elif gate == "i":
            i_t = gates.tile([128, H], F32, tag="gact")
            nc.scalar.activation(i_t, z[:, :], AF.Sigmoid)
        elif gate == "f":
            f_t = gates.tile([128, H], F32, tag="gact")
            nc.scalar.activation(f_t, z[:, :], AF.Sigmoid)
        else:
            o_t = gates.tile([128, H], F32, tag="gact")
            nc.scalar.activation(o_t, z[:, :], AF.Sigmoid)

    # c = f*c + i*g   (folded layout, full 128 partitions)
    ig = temps.tile([128, H], F32, tag="tmp")
    nc.vector.tensor_mul(ig, i_t, g_t)
    if first:
        nc.vector.tensor_copy(c_fold[:, :], ig)
    else:
        fc = temps.tile([128, H], F32, tag="tmp")
        nc.vector.tensor_mul(fc, f_t, c_fold[:, :])
        nc.vector.tensor_add(c_fold[:, :], fc, ig)

    tc_t = gates.tile([128, H], F32, tag="gact")
    nc.scalar.activation(tc_t, c_fold[:, :], AF.Tanh)

    h_fold = pools["hfold"].tile([128, H], BF16, tag="hfold")
    nc.vector.tensor_mul(h_fold, o_t, tc_t)
    return h_fold


def _emit_transpose_h(nc, pools, h_fold, idn2):
    """h_fold [128, 512] bf16 -> hT [128, NK, 64] bf16 via 16 [64,64] PE
    transposes. Full-width (128-col) transposes with mixed row-groups into
    one PSUM bank fault the device, so each k-tile is built from two
    half-blocks with explicit (row, col) tile positions."""
    hps = pools["psum_h"].tile([128, NK, BS], BF16, tag="hps")
    for k in range(NK):
        half, kk = divmod(k, 4)
        p0 = 64 * half
        for cg in (0, 64):
            nc.tensor.transpose(
                hps[cg:cg + 64, k, :],
                h_fold[p0:p0 + 64, 128 * kk + cg:128 * kk + cg + 64],
                idn2[p0:p0 + 64, :],
                tile_position=(p0, cg),
            )
    hT = pools["hT"].tile([128, NK, BS], BF16, tag="hT")
    nc.vector.tensor_copy(hT[:, :, :], hps[:, :, :])
    return hT


def _emit_pred(nc, pools, wd_sb, hT, out_dram, step_idx, bd_sb=None):
    """predT = Wd.T @ hT  -> DMA to out[step_idx]; returns predT bf16 tile."""
    pd = pools["psum_p"].tile([128, BS], F32, tag="pd")
    for k in range(NK):
        nc.tensor.matmul(
            pd[:, :], wd_sb[:, k, :], hT[:, k, :],
            start=(k == 0), stop=(k == NK - 1),
        )
    pdv = pools["temps"].tile([128, BS], F32, tag="pdv")
    if bd_sb is not None:
        nc.vector.tensor_scalar_add(pdv, pd[:, :], bd_sb[:, 0:1])
    else:
        nc.vector.tensor_copy(pdv[:, :], pd[:, :])
    nc.sync.dma_start(out=out_dram[step_idx], in_=pdv[:, :])
    predT = pools["predT"].tile([128, BS], BF16, tag="predT")
    nc.vector.tensor_copy(predT[:, :], pdv[:, :])
    return predT


def _build(has_b1, has_b2, has_bd):
    nc = bacc.Bacc("TRN2", target_bir_lowering=False, debug=False)

    XT = nc.declare_dram_parameter("xt", [F, T, BS], BF16, isOutput=False)
    U1 = nc.declare_dram_parameter("u1", [128, NK, G], BF16, isOutput=False)
    W1 = nc.declare_dram_parameter("w1", [F, G], BF16, isOutput=False)
    W2 = nc.declare_dram_parameter("w2", [128, NK, G], BF16, isOutput=False)
    WD = nc.declare_dram_parameter("wd", [128, NK, F], BF16, isOutput=False)
    IDN = nc.declare_dram_parameter("idn", [128, 64], BF16, isOutput=False)
    if has_b1:
        B1 = nc.declare_dram_parameter("b1f", [4, 128, H], F32, isOutput=False)
    if has_b2:
        B2 = nc.declare_dram_parameter("b2f", [4, 128, H], F32, isOutput=False)
    if has_bd:
        BD = nc.declare_dram_parameter("bdf", [128, 1], F32, isOutput=False)
    OUT = nc.declare_dram_parameter("out", [OUT_STEPS, F, BS], F32, isOutput=True)

    with tile.TileContext(nc) as tc, ExitStack() as ctx:
        consts = ctx.enter_context(tc.tile_pool(name="consts", bufs=1))
        pools = {
            "psum": ctx.enter_context(tc.tile_pool(name="psum", bufs=5, space="PSUM")),
            "psum_h": ctx.enter_context(tc.tile_pool(name="psum_h", bufs=2, space="PSUM")),
            "psum_p": ctx.enter_context(tc.tile_pool(name="psum_p", bufs=1, space="PSUM")),
            "gates": ctx.enter_context(tc.tile_pool(name="gates", bufs=8)),
            "temps": ctx.enter_context(tc.tile_pool(name="temps", bufs=4)),
            "hfold": ctx.enter_context(tc.tile_pool(name="hfold", bufs=2)),
            "hT": ctx.enter_context(tc.tile_pool(name="hT", bufs=2)),
            "predT": ctx.enter_context(tc.tile_pool(name="predT", bufs=2)),
        }

        xt_sb = consts.tile([F, T, BS], BF16)
        u1_sb = consts.tile([128, NK, G], BF16)
        w1_sb = consts.tile([F, G], BF16)
        w2_sb = consts.tile([128, NK, G], BF16)
        wd_sb = consts.tile([128, NK, F], BF16)
        idn_sb = consts.tile([128, 64], BF16)
        nc.sync.dma_start(out=xt_sb[:], in_=XT[:])
        nc.sync.dma_start(out=w1_sb[:], in_=W1[:])
        for k in range(NK):  # split big weight DMAs so early steps start sooner
            nc.sync.dma_start(out=u1_sb[:, k, :], in_=U1[:, k, :])
        for k in range(NK):
            nc.sync.dma_start(out=w2_sb[:, k, :], in_=W2[:, k, :])
        nc.sync.dma_start(out=wd_sb[:], in_=WD[:])
        nc.sync.dma_start(out=idn_sb[:], in_=IDN[:])

        b1_tiles = b2_tiles = None
        if has_b1:
            b1_sb = consts.tile([4, 128, H], F32)
            nc.sync.dma_start(out=b1_sb[:], in_=B1[:])
            b1_tiles = {g: b1_sb[i] for i, g in enumerate(("i", "f", "g", "o"))}
        if has_b2:
            b2_sb = consts.tile([4, 128, H], F32)
            nc.sync.dma_start(out=b2_sb[:], in_=B2[:])
            b2_tiles = {g: b2_sb[i] for i, g in enumerate(("i", "f", "g", "o"))}
        bd_sb = None
        if has_bd:
            bd_sb = consts.tile([128, 1], F32)
            nc.sync.dma_start(out=bd_sb[:], in_=BD[:])

        c_fold = consts.tile([128, H], F32)  # persistent cell state

        # ---- warmup: 64 steps of cell1 over the input sequence ----
        hT = None
        for t in range(T):
            if t == 0:
                stats = [xt_sb[:, 0, :]]
                rhs = lambda k, off: w1_sb[:, off:off + H]
            else:
                stats = [xt_sb[:, t, :]] + [hT[:, k, :] for k in range(NK)]
                rhs = (lambda t_: lambda k, off:
                       w1_sb[:, off:off + H] if k == 0
                       else u1_sb[:, k - 1, off:off + H])(t)
            h_fold = _emit_cell(nc, pools, stats, rhs, c_fold, first=(t == 0),
                                bias_tile=b1_tiles)
            hT = _emit_transpose_h(nc, pools, h_fold, idn_sb)

        # ---- pred0 ----
        predT = _emit_pred(nc, pools, wd_sb, hT, OUT, 0, bd_sb)

        # ---- autoregressive: 31 steps of cell1 + cell2 ----
        for t in range(OUT_STEPS - 1):
            stats1 = [predT] + [hT[:, k, :] for k in range(NK)]
            rhs1 = lambda k, off: (w1_sb[:, off:off + H] if k == 0
                                   else u1_sb[:, k - 1, off:off + H])
            h1_fold = _emit_cell(nc, pools, stats1, rhs1, c_fold, first=False,
                                 bias_tile=b1_tiles)
            h1T = _emit_transpose_h(nc, pools, h1_fold, idn_sb)

            stats2 = [h1T[:, k, :] for k in range(NK)]
            rhs2 = lambda k, off: w2_sb[:, k, off:off + H]
            h_fold = _emit_cell(nc, pools, stats2, rhs2, c_fold, first=False,
                                bias_tile=b2_tiles)
            hT = _emit_transpose_h(nc, pools, h_fold, idn_sb)

            predT = _emit_pred(nc, pools, wd_sb, hT, OUT, t + 1, bd_sb)

    nc.compile()
    return nc


def _fold_bias(b):
    """[4096] gate bias -> [4, 128, 512] folded tiles in (i,f,g,o) order."""
    out = np.zeros((4, 128, H), np.float32)
    for gi, gname in enumerate(("i", "f", "g", "o")):
        off = GATE_OFF[gname]
        out[gi, 0:64, :] = b[off:off + H][None, :]
        out[gi, 64:128, :] = b[off + H:off + 2 * H][None, :]
    return out


def kernel(inputs, mean, var, W1, U1, b1, W2, U2, b2, Wd, bd):
    x = np.asarray(inputs, np.float32)
    mean = np.asarray(mean, np.float32)
    var = np.asarray(var, np.float32)
    inv = 1.0 / np.sqrt(var + EPS)
    xn = ((x - mean) * inv - mean) * inv  # reference normalizes twice

    W1 = np.asarray(W1, np.float32)
    U1k = np.asarray(U1, np.float32).reshape(NK, 128, G).transpose(1, 0, 2)
    W2U2 = (np.asarray(W2, np.float32) + np.asarray(U2, np.float32))
    W2k = W2U2.reshape(NK, 128, G).transpose(1, 0, 2)
    WDk = np.asarray(Wd, np.float32).reshape(NK, 128, F).transpose(1, 0, 2)
    idn2 = np.zeros((128, 64), np.float32)
    idn2[0:64] = np.eye(64)
    idn2[64:128] = np.eye(64)

    b1 = np.asarray(b1, np.float32)
    b2 = np.asarray(b2, np.float32)
    bd = np.asarray(bd, np.float32)
    has_b1 = bool(np.any(b1))
    has_b2 = bool(np.any(b2))
    has_bd = bool(np.any(bd))

    key = (has_b1, has_b2, has_bd)
    if key not in _BUILD_CACHE:
        _BUILD_CACHE[key] = _build(*key)
    nc = _BUILD_CACHE[key]

    bf = ml_dtypes.bfloat16
    shared = {
        "u1": np.ascontiguousarray(U1k).astype(bf),
        "w1": W1.astype(bf),
        "w2": np.ascontiguousarray(W2k).astype(bf),
        "wd": np.ascontiguousarray(WDk).astype(bf),
        "idn": idn2.astype(bf),
    }
    if has_b1:
        shared["b1f"] = _fold_bias(b1)
    if has_b2:
        shared["b2f"] = _fold_bias(b2)
    if has_bd:
        shared["bdf"] = bd.reshape(128, 1).astype(np.float32)

    in_maps = []
    for c in range(NCORES):
        shard = xn[c * BS:(c + 1) * BS]              # [64, 64, 128]
        xt = np.ascontiguousarray(shard.transpose(2, 1, 0)).astype(bf)
        m = dict(shared)
        m["xt"] = xt
        in_maps.append(m)

    res = run_bass_kernel_spmd(nc, in_maps, core_ids=list(range(NCORES)))
    kernel.last_results = res

    # per-core out: [32, 128 feat, 64 batch] -> [64, 32, 128]
    parts = [res.results[c]["out"].transpose(2, 0, 1) for c in range(NCORES)]
    return np.ascontiguousarray(np.concatenate(parts, axis=0), dtype=np.float32)


# revision 8
# speedup vs baseline: 1.0345x; 1.0345x over previous
"""Trainium2 Bass kernel for the AutoRegressiveLSTM problem.

Strategy: data-parallel over batch (512 -> 64 rows per NeuronCore, 8 cores,
zero inter-core communication). All weights resident in SBUF as bf16;
matmuls in bf16 with fp32 PSUM accumulation (validated ~4e-3 max rel err
through the full 95-step recurrence).

Per-core layouts:
  - LSTM state h is kept TRANSPOSED (hT, [unit, batch]) because the
    TensorEngine computes out = lhsT.T @ rhs: z[batch, gates] needs
    stationary hT k-tiles [128 units, 64 batch].
  - Gate pre-activations z land in PSUM "gate-folded": each [128, 512]
    PSUM tile holds one gate, partitions 0:64 = units 0:512 (batch-major),
    partitions 64:128 = units 512:1024. The two halves are two independent
    matmul accumulation chains targeting different PE column groups, which
    the hardware runs concurrently (recovers full 128-wide array
    utilization despite the 64-row batch shard).
  - c / h state stays in the same folded [128, 512] layout, so all
    elementwise ops run at full 128-partition width.
  - h is un-folded back to hT via 8 PE transposes per cell.
  - pred (the Dense output) is computed transposed (predT = Wd.T @ hT) and
    written per-step to DRAM as [128 feat, 64 batch]; the host unshards.

The double normalization of the input and W2+U2 (cell2 sees x == h) are
folded on the host.
"""

from contextlib import ExitStack

import numpy as np
import ml_dtypes

import concourse.bass as bass
import concourse.tile as tile
from concourse import bacc, mybir
from concourse.bass_utils import run_bass_kernel_spmd

BF16 = mybir.dt.bfloat16
F32 = mybir.dt.float32
AF = mybir.ActivationFunctionType

NCORES = 8
B_FULL = 512
BS = B_FULL // NCORES   # 64 batch rows per core
T = 64                  # warmup sequence length
F = 128                 # features
U = 1024                # LSTM units
G = 4 * U               # 4096 gate columns
NK = U // 128           # 8 contraction k-tiles
OUT_STEPS = 32
EPS = 1e-7

# gate column ranges in the natural [i f g o] weight layout, split in two
# 512-wide halves; each pair (lo, hi) is one gate's two halves and maps to
# PSUM partitions [0:64] / [64:128] of one [128, 512] tile.
GATE_OFF = {"i": 0, "f": U, "g": 2 * U, "o": 3 * U}
H = 512  # half-gate width

_BUILD_CACHE = {}


def _emit_cell(nc, pools, stats, rhs_of, c_fold, first, idn2, bias_tile=None):
    """One LSTM cell: z = sum_k stats[k].T @ rhs_of(k, col) for all four
    gates, activations, state update, and un-fold of h back to transposed
    layout. Returns hT [128, NK, 64] bf16.

    The post-matmul tail (sigmoid(o) -> h -> transpose -> copy) is
    pipelined in two 256-column slices to keep the PE fed and the
    HAM clock-gate warm.

    stats: list of stationary APs ([128, <=128] bf16 k-tiles)
    rhs_of: (k, col_off) -> moving AP [128, 512]
    c_fold: persistent cell-state tile [128, 512] f32
    first: if True, h==0 and c==0 on entry (skip f*c, direct init)
    """
    psum, gates, temps = pools["psum"], pools["gates"], pools["temps"]
    nk = len(stats)
    # order: g, i, f, o  (c needs g,i,f; o only feeds the output tail)
    zp = {}
    acts = {}
    for gate in ("g", "i", "f", "o"):
        off = GATE_OFF[gate]
        z = psum.tile([128, H], F32, tag="zp")
        for k in range(nk):
            # the two chains write disjoint partition halves of one bank;
            # the sim's zero-region group check is partition-blind, so skip
            nc.tensor.matmul(
                z[0:64, :], stats[k], rhs_of(k, off),
                start=(k == 0), stop=(k == nk - 1), skip_group_check=True,
            )
            nc.tensor.matmul(
                z[64:128, :], stats[k], rhs_of(k, off + H),
                start=(k == 0), stop=(k == nk - 1), skip_group_check=True,
            )
        if bias_tile is not None:
            nc.vector.tensor_add(z[:, :], z[:, :], bias_tile[gate])
        zp[gate] = z
        if gate == "o":
            continue  # o activated per-slice in the tail below
        a = gates.tile([128, H], F32, tag="gact")
        nc.scalar.activation(a, z[:, :], AF.Tanh if gate == "g" else AF.Sigmoid)
        acts[gate] = a

    h_fold = pools["hfold"].tile([128, H], BF16, tag="hfold")
    hps = pools["psum_h"].tile([128, NK, BS], BF16, tag="hps")
    hT = pools["hT"].tile([128, NK, BS], BF16, tag="hT")

    HS = H // 2  # 256-wide tail slices
    for s in (0, 1):
        sl = slice(HS * s, HS * (s + 1))
        ig = temps.tile([128, HS], F32, tag="tmp")
        nc.vector.tensor_mul(ig, acts["i"][:, sl], acts["g"][:, sl])
        if first:
            nc.vector.tensor_copy(c_fold[:, sl], ig)
        else:
            fc = temps.tile([128, HS], F32, tag="tmp")
            nc.vector.tensor_mul(fc, acts["f"][:, sl], c_fold[:, sl])
            nc.vector.tensor_add(c_fold[:, sl], fc, ig)
        tc_t = gates.tile([128, HS], F32, tag="tcs")
        nc.scalar.activation(tc_t, c_fold[:, sl], AF.Tanh)
        o_t = gates.tile([128, HS], F32, tag="tcs")
        nc.scalar.activation(o_t, zp["o"][:, sl], AF.Sigmoid)
        nc.vector.tensor_mul(h_fold[:, sl], o_t, tc_t)

        # un-fold this slice: cols [256s, 256s+256) are unit blocks
        # kk in {2s, 2s+1} of both partition halves. 16 [64,64] transposes
        # per cell (full-width transposes with mixed row-groups into one
        # PSUM bank fault the device).
        for kk in (2 * s, 2 * s + 1):
            for half in (0, 1):
                p0 = 64 * half
                k = 4 * half + kk
                for cg in (0, 64):
                    nc.tensor.transpose(
                        hps[cg:cg + 64, k, :],
                        h_fold[p0:p0 + 64, 128 * kk + cg:128 * kk + cg + 64],
                        idn2[p0:p0 + 64, :],
                        tile_position=(p0, cg),
                    )
        nc.vector.tensor_copy(hT[:, 2 * s:2 * s + 2, :], hps[:, 2 * s:2 * s + 2, :])
        nc.vector.tensor_copy(hT[:, 4 + 2 * s:6 + 2 * s, :], hps[:, 4 + 2 * s:6 + 2 * s, :])
    return hT


def _emit_pred(nc, pools, wd_sb, hT, out_dram, step_idx, bd_sb=None):
    """predT = Wd.T @ hT  -> DMA to out[step_idx]; returns predT bf16 tile."""
    pd = pools["psum_p"].tile([128, BS], F32, tag="pd")
    for k in range(NK):
        nc.tensor.matmul(
            pd[:, :], wd_sb[:, k, :], hT[:, k, :],
            start=(k == 0), stop=(k == NK - 1),
        )
    pdv = pools["temps"].tile([128, BS], F32, tag="pdv")
    if bd_sb is not None:
        nc.vector.tensor_scalar_add(pdv, pd[:, :], bd_sb[:, 0:1])
    else:
        nc.vector.tensor_copy(pdv[:, :], pd[:, :])
    nc.sync.dma_start(out=out_dram[step_idx], in_=pdv[:, :])
    predT = pools["predT"].tile([128, BS], BF16, tag="predT")
    nc.vector.tensor_copy(predT[:, :], pdv[:, :])
    return predT


def _build(has_b1, has_b2, has_bd):
    nc = bacc.Bacc("TRN2", target_bir_lowering=False, debug=False)

    XT = nc.declare_dram_parameter("xt", [F, T, BS], BF16, isOutput=False)
    U1 = nc.declare_dram_parameter("u1", [128, NK, G], BF16, isOutput=False)
    W1 = nc.declare_dram_parameter("w1", [F, G], BF16, isOutput=False)
    W2 = nc.declare_dram_parameter("w2", [128, NK, G], BF16, isOutput=False)
    WD = nc.declare_dram_parameter("wd", [128, NK, F], BF16, isOutput=False)
    IDN = nc.declare_dram_parameter("idn", [128, 64], BF16, isOutput=False)
    if has_b1:
        B1 = nc.declare_dram_parameter("b1f", [4, 128, H], F32, isOutput=False)
    if has_b2:
        B2 = nc.declare_dram_parameter("b2f", [4, 128, H], F32, isOutput=False)
    if has_bd:
        BD = nc.declare_dram_parameter("bdf", [128, 1], F32, isOutput=False)
    OUT = nc.declare_dram_parameter("out", [OUT_STEPS, F, BS], F32, isOutput=True)

    with tile.TileContext(nc) as tc, ExitStack() as ctx:
        consts = ctx.enter_context(tc.tile_pool(name="consts", bufs=1))
        pools = {
            "psum": ctx.enter_context(tc.tile_pool(name="psum", bufs=5, space="PSUM")),
            "psum_h": ctx.enter_context(tc.tile_pool(name="psum_h", bufs=2, space="PSUM")),
            "psum_p": ctx.enter_context(tc.tile_pool(name="psum_p", bufs=1, space="PSUM")),
            "gates": ctx.enter_context(tc.tile_pool(name="gates", bufs=8)),
            "temps": ctx.enter_context(tc.tile_pool(name="temps", bufs=4)),
            "hfold": ctx.enter_context(tc.tile_pool(name="hfold", bufs=2)),
            "hT": ctx.enter_context(tc.tile_pool(name="hT", bufs=2)),
            "predT": ctx.enter_context(tc.tile_pool(name="predT", bufs=2)),
        }

        xt_sb = consts.tile([F, T, BS], BF16)
        u1_sb = consts.tile([128, NK, G], BF16)
        w1_sb = consts.tile([F, G], BF16)
        w2_sb = consts.tile([128, NK, G], BF16)
        wd_sb = consts.tile([128, NK, F], BF16)
        idn_sb = consts.tile([128, 64], BF16)
        nc.sync.dma_start(out=xt_sb[:], in_=XT[:])
        nc.sync.dma_start(out=w1_sb[:], in_=W1[:])
        for k in range(NK):  # split big weight DMAs so early steps start sooner
            nc.sync.dma_start(out=u1_sb[:, k, :], in_=U1[:, k, :])
        for k in range(NK):
            nc.sync.dma_start(out=w2_sb[:, k, :], in_=W2[:, k, :])
        nc.sync.dma_start(out=wd_sb[:], in_=WD[:])
        nc.sync.dma_start(out=idn_sb[:], in_=IDN[:])

        b1_tiles = b2_tiles = None
        if has_b1:
            b1_sb = consts.tile([4, 128, H], F32)
            nc.sync.dma_start(out=b1_sb[:], in_=B1[:])
            b1_tiles = {g: b1_sb[i] for i, g in enumerate(("i", "f", "g", "o"))}
        if has_b2:
            b2_sb = consts.tile([4, 128, H], F32)
            nc.sync.dma_start(out=b2_sb[:], in_=B2[:])
            b2_tiles = {g: b2_sb[i] for i, g in enumerate(("i", "f", "g", "o"))}
        bd_sb = None
        if has_bd:
            bd_sb = consts.tile([128, 1], F32)
            nc.sync.dma_start(out=bd_sb[:], in_=BD[:])

        c_fold = consts.tile([128, H], F32)  # persistent cell state

        # ---- warmup: 64 steps of cell1 over the input sequence ----
        hT = None
        for t in range(T):
            if t == 0:
                stats = [xt_sb[:, 0, :]]
                rhs = lambda k, off: w1_sb[:, off:off + H]
            else:
                stats = [xt_sb[:, t, :]] + [hT[:, k, :] for k in range(NK)]
                rhs = (lambda t_: lambda k, off:
                       w1_sb[:, off:off + H] if k == 0
                       else u1_sb[:, k - 1, off:off + H])(t)
            hT = _emit_cell(nc, pools, stats, rhs, c_fold, first=(t == 0),
                            idn2=idn_sb, bias_tile=b1_tiles)

        # ---- pred0 ----
        predT = _emit_pred(nc, pools, wd_sb, hT, OUT, 0, bd_sb)

        # ---- autoregressive: 31 steps of cell1 + cell2 ----
        for t in range(OUT_STEPS - 1):
            stats1 = [predT] + [hT[:, k, :] for k in range(NK)]
            rhs1 = lambda k, off: (w1_sb[:, off:off + H] if k == 0
                                   else u1_sb[:, k - 1, off:off + H])
            h1T = _emit_cell(nc, pools, stats1, rhs1, c_fold, first=False,
                             idn2=idn_sb, bias_tile=b1_tiles)

            stats2 = [h1T[:, k, :] for k in range(NK)]
            rhs2 = lambda k, off: w2_sb[:, k, off:off + H]
            hT = _emit_cell(nc, pools, stats2, rhs2, c_fold, first=False,
                            idn2=idn_sb, bias_tile=b2_tiles)

            predT = _emit_pred(nc, pools, wd_sb, hT, OUT, t + 1, bd_sb)

    nc.compile()
    return nc


def _fold_bias(b):
    """[4096] gate bias -> [4, 128, 512] folded tiles in (i,f,g,o) order."""
    out = np.zeros((4, 128, H), np.float32)
    for gi, gname in enumerate(("i", "f", "g", "o")):
        off = GATE_OFF[gname]
        out[gi, 0:64, :] = b[off:off + H][None, :]
        out[gi, 64:128, :] = b[off + H:off + 2 * H][None, :]
    return out


def kernel(inputs, mean, var, W1, U1, b1, W2, U2, b2, Wd, bd):
    x = np.asarray(inputs, np.float32)
    mean = np.asarray(mean, np.float32)
    var = np.asarray(var, np.float32)
    inv = 1.0 / np.sqrt(var + EPS)
    xn = ((x - mean) * inv - mean) * inv  # reference normalizes twice

    W1 = np.asarray(W1, np.float32)
    U1k = np.asarray(U1, np.float32).reshape(NK, 128, G).transpose(1, 0, 2)
    W2U2 = (np.asarray(W2, np.float32) + np.asarray(U2, np.float32))
    W2k = W2U2.reshape(NK, 128, G).transpose(1, 0, 2)
    WDk = np.asarray(Wd, np.float32).reshape(NK, 128, F).transpose(1, 0, 2)
    idn2 = np.zeros((128, 64), np.float32)
    idn2[0:64] = np.eye(64)
    idn2[64:128] = np.eye(64)

    b1 = np.asarray(b1, np.float32)
    b2 = np.asarray(b2, np.float32)
    bd = np.asarray(bd, np.float32)
    has_b1 = bool(np.any(b1))
    has_b2 = bool(np.any(b2))
    has_bd = bool(np.any(bd))

    key = (has_b1, has_b2, has_bd)
    if key not in _BUILD_CACHE:
        _BUILD_CACHE[key] = _build(*key)
    nc = _BUILD_CACHE[key]

    bf = ml_dtypes.bfloat16
    shared = {
        "u1": np.ascontiguousarray(U1k).astype(bf),
        "w1": W1.astype(bf),
        "w2": np.ascontiguousarray(W2k).astype(bf),
        "wd": np.ascontiguousarray(WDk).astype(bf),
        "idn": idn2.astype(bf),
    }
    if has_b1:
        shared["b1f"] = _fold_bias(b1)
    if has_b2:
        shared["b2f"] = _fold_bias(b2)
    if has_bd:
        shared["bdf"] = bd.reshape(128, 1).astype(np.float32)

    in_maps = []
    for c in range(NCORES):
        shard = xn[c * BS:(c + 1) * BS]              # [64, 64, 128]
        xt = np.ascontiguousarray(shard.transpose(2, 1, 0)).astype(bf)
        m = dict(shared)
        m["xt"] = xt
        in_maps.append(m)

    res = run_bass_kernel_spmd(nc, in_maps, core_ids=list(range(NCORES)))
    kernel.last_results = res

    # per-core out: [32, 128 feat, 64 batch] -> [64, 32, 128]
    parts = [res.results[c]["out"].transpose(2, 0, 1) for c in range(NCORES)]
    return np.ascontiguousarray(np.concatenate(parts, axis=0), dtype=np.float32)


# revision 12
# speedup vs baseline: 1.0952x; 1.0587x over previous
"""Trainium2 Bass kernel for the AutoRegressiveLSTM problem.

Strategy: data-parallel over batch (512 -> 64 rows per NeuronCore, 8 cores,
zero inter-core communication). All weights resident in SBUF as bf16;
matmuls in bf16 with fp32 PSUM accumulation (validated ~4e-3 max rel err
through the full 95-step recurrence).

Per-core layouts:
  - LSTM state h is kept TRANSPOSED (hT, [unit, batch]) because the
    TensorEngine computes out = lhsT.T @ rhs: z[batch, gates] needs
    stationary hT k-tiles [128 units, 64 batch].
  - Gate pre-activations z land in PSUM "gate-folded": each [128, 512]
    PSUM tile holds one gate, partitions 0:64 = units 0:512 (batch-major),
    partitions 64:128 = units 512:1024. The two halves are two independent
    matmul accumulation chains targeting different PE column groups, which
    the hardware runs concurrently (recovers full 128-wide array
    utilization despite the 64-row batch shard).
  - c / h state stays in the same folded [128, 512] layout, so all
    elementwise ops run at full 128-partition width.
  - h is un-folded back to hT via 8 PE transposes per cell.
  - pred (the Dense output) is computed transposed (predT = Wd.T @ hT) and
    written per-step to DRAM as [128 feat, 64 batch]; the host unshards.

The double normalization of the input and W2+U2 (cell2 sees x == h) are
folded on the host.
"""

from contextlib import ExitStack

import numpy as np
import ml_dtypes

import concourse.bass as bass
import concourse.tile as tile
from concourse import bacc, mybir
from concourse.bass_utils import run_bass_kernel_spmd

BF16 = mybir.dt.bfloat16
F32 = mybir.dt.float32
AF = mybir.ActivationFunctionType

NCORES = 8
B_FULL = 512
BS = B_FULL // NCORES   # 64 batch rows per core
T = 64                  # warmup sequence length
F = 128                 # features
U = 1024                # LSTM units
G = 4 * U               # 4096 gate columns
NK = U // 128           # 8 contraction k-tiles
OUT_STEPS = 32
EPS = 1e-7

# gate column ranges in the natural [i f g o] weight layout, split in two
# 512-wide halves; each pair (lo, hi) is one gate's two halves and maps to
# PSUM partitions [0:64] / [64:128] of one [128, 512] tile.
GATE_OFF = {"i": 0, "f": U, "g": 2 * U, "o": 3 * U}
H = 512  # half-gate width

_BUILD_CACHE = {}


def _emit_chains(nc, pools, terms, zp=None, final=True):
    """Emit the paired matmul accumulation chains for all four gates.

    terms: list of (stationary AP, rhs_fn(col_off) -> moving AP), appended
    to the (possibly pre-started) accumulation groups in `zp`.
    Returns the zp dict {gate: psum tile [128, 512]}.
    """
    psum = pools["psum"]
    new = zp is None
    if new:
        zp = {g: psum.tile([128, H], F32, name=f"z_{g}", tag="zp") for g in ("g", "i", "f", "o")}
    nt = len(terms)
    for gate in ("g", "i", "f", "o"):
        off = GATE_OFF[gate]
        z = zp[gate]
        for j, (stat, rf) in enumerate(terms):
            start = new and j == 0
            stop = final and j == nt - 1
            # the two chains write disjoint partition halves of one bank;
            # the sim's zero-region group check is partition-blind, so skip
            nc.tensor.matmul(
                z[0:64, :], stat, rf(off),
                start=start, stop=stop, skip_group_check=True,
            )
            nc.tensor.matmul(
                z[64:128, :], stat, rf(off + H),
                start=start, stop=stop, skip_group_check=True,
            )
    return zp


NSL = 4            # tail slices
SW = H // NSL      # 128 cols per slice


def _emit_cell(nc, pools, zp, c_fold, first, idn2, bias_tile=None):
    """Activations + state update for one LSTM cell whose gate matmuls are
    already emitted into `zp`. Returns (hT tile, tail_fn).

    Everything that can run during the o-gate matmul stream (g/i/f
    activations, c update, tanh(c)) is emitted inline. tail_fn emits the
    o-dependent tail (sigmoid(o) -> h -> transposes -> hT copies) in
    NSL pipelined slices; the caller can pre-issue independent PE work
    (the next cell's x-side matmuls) before invoking it so the in-order
    PE queue stays fed.
    """
    gates, temps = pools["gates"], pools["temps"]
    if bias_tile is not None:
        for gate in ("g", "i", "f", "o"):
            nc.vector.tensor_add(zp[gate][:, :], zp[gate][:, :], bias_tile[gate])
    acts = {}
    for gate in ("g", "i", "f"):
        a = gates.tile([128, H], F32, tag="gact")
        nc.scalar.activation(a, zp[gate][:, :], AF.Tanh if gate == "g" else AF.Sigmoid)
        acts[gate] = a

    tc_s = []
    for s in range(NSL):
        sl = slice(SW * s, SW * (s + 1))
        ig = temps.tile([128, SW], F32, tag="tmp")
        nc.vector.tensor_mul(ig, acts["i"][:, sl], acts["g"][:, sl])
        if first:
            nc.vector.tensor_copy(c_fold[:, sl], ig)
        else:
            fc = temps.tile([128, SW], F32, tag="tmp")
            nc.vector.tensor_mul(fc, acts["f"][:, sl], c_fold[:, sl])
            nc.vector.tensor_add(c_fold[:, sl], fc, ig)
        t = gates.tile([128, SW], F32, tag="tcs")
        nc.scalar.activation(t, c_fold[:, sl], AF.Tanh)
        tc_s.append(t)

    h_fold = pools["hfold"].tile([128, H], BF16, tag="hfold")
    hT = pools["hT"].tile([128, NK, BS], BF16, tag="hT")

    def tail():
        hpsA = pools["psum_hA"].tile([128, NSL, BS], BF16, tag="hpsA")
        hpsB = pools["psum_hB"].tile([128, NSL, BS], BF16, tag="hpsB")
        for s in range(NSL):
            sl = slice(SW * s, SW * (s + 1))
            o_t = gates.tile([128, SW], F32, tag="tcs")
            nc.scalar.activation(o_t, zp["o"][:, sl], AF.Sigmoid)
            nc.vector.tensor_mul(h_fold[:, sl], o_t, tc_s[s])
            # full-width transposes; each PSUM bank sees only one row-group
            # (mixed row-groups into one bank fault the device)
            nc.tensor.transpose(hpsA[:, s, :], h_fold[0:64, sl], idn2[0:64, :])
            nc.tensor.transpose(hpsB[:, s, :], h_fold[64:128, sl], idn2[64:128, :])
        nc.vector.tensor_copy(hT[:, 0:NSL, :], hpsA[:, :, :])
        nc.vector.tensor_copy(hT[:, NSL:NK, :], hpsB[:, :, :])

    return hT, tail



def _emit_pred(nc, pools, wd_sb, hT, out_dram, step_idx, bd_sb=None):
    """predT = Wd.T @ hT  -> DMA to out[step_idx]; returns predT bf16 tile."""
    pd = pools["psum_p"].tile([128, BS], F32, tag="pd")
    for k in range(NK):
        nc.tensor.matmul(
            pd[:, :], wd_sb[:, k, :], hT[:, k, :],
            start=(k == 0), stop=(k == NK - 1),
        )
    pdv = pools["temps"].tile([128, BS], F32, tag="pdv")
    if bd_sb is not None:
        nc.vector.tensor_scalar_add(pdv, pd[:, :], bd_sb[:, 0:1])
    else:
        nc.vector.tensor_copy(pdv[:, :], pd[:, :])
    nc.sync.dma_start(out=out_dram[step_idx], in_=pdv[:, :])
    predT = pools["predT"].tile([128, BS], BF16, tag="predT")
    nc.vector.tensor_copy(predT[:, :], pdv[:, :])
    return predT


def _build(has_b1, has_b2, has_bd):
    nc = bacc.Bacc("TRN2", target_bir_lowering=False, debug=False)

    XT = nc.declare_dram_parameter("xt", [F, T, BS], BF16, isOutput=False)
    U1 = nc.declare_dram_parameter("u1", [128, NK, G], BF16, isOutput=False)
    W1 = nc.declare_dram_parameter("w1", [F, G], BF16, isOutput=False)
    W2 = nc.declare_dram_parameter("w2", [128, NK, G], BF16, isOutput=False)
    WD = nc.declare_dram_parameter("wd", [128, NK, F], BF16, isOutput=False)
    IDN = nc.declare_dram_parameter("idn", [128, 64], BF16, isOutput=False)
    if has_b1:
        B1 = nc.declare_dram_parameter("b1f", [4, 128, H], F32, isOutput=False)
    if has_b2:
        B2 = nc.declare_dram_parameter("b2f", [4, 128, H], F32, isOutput=False)
    if has_bd:
        BD = nc.declare_dram_parameter("bdf", [128, 1], F32, isOutput=False)
    OUT = nc.declare_dram_parameter("out", [OUT_STEPS, F, BS], F32, isOutput=True)

    with tile.TileContext(nc) as tc, ExitStack() as ctx:
        consts = ctx.enter_context(tc.tile_pool(name="consts", bufs=1))
        pools = {
            "psum": ctx.enter_context(tc.tile_pool(name="psum", bufs=5, space="PSUM")),
            "psum_hA": ctx.enter_context(tc.tile_pool(name="psum_hA", bufs=1, space="PSUM")),
            "psum_hB": ctx.enter_context(tc.tile_pool(name="psum_hB", bufs=1, space="PSUM")),
            "psum_p": ctx.enter_context(tc.tile_pool(name="psum_p", bufs=1, space="PSUM")),
            "gates": ctx.enter_context(tc.tile_pool(name="gates", bufs=8)),
            "temps": ctx.enter_context(tc.tile_pool(name="temps", bufs=4)),
            "hfold": ctx.enter_context(tc.tile_pool(name="hfold", bufs=2)),
            "hT": ctx.enter_context(tc.tile_pool(name="hT", bufs=2)),
            "predT": ctx.enter_context(tc.tile_pool(name="predT", bufs=2)),
        }

        xt_sb = consts.tile([F, T, BS], BF16)
        u1_sb = consts.tile([128, NK, G], BF16)
        w1_sb = consts.tile([F, G], BF16)
        w2_sb = consts.tile([128, NK, G], BF16)
        wd_sb = consts.tile([128, NK, F], BF16)
        idn_sb = consts.tile([128, 64], BF16)
        nc.sync.dma_start(out=xt_sb[:], in_=XT[:])
        nc.sync.dma_start(out=w1_sb[:], in_=W1[:])
        for k in range(NK):  # split big weight DMAs so early steps start sooner
            nc.sync.dma_start(out=u1_sb[:, k, :], in_=U1[:, k, :])
        for k in range(NK):
            nc.sync.dma_start(out=w2_sb[:, k, :], in_=W2[:, k, :])
        nc.sync.dma_start(out=wd_sb[:], in_=WD[:])
        nc.sync.dma_start(out=idn_sb[:], in_=IDN[:])

        b1_tiles = b2_tiles = None
        if has_b1:
            b1_sb = consts.tile([4, 128, H], F32)
            nc.sync.dma_start(out=b1_sb[:], in_=B1[:])
            b1_tiles = {g: b1_sb[i] for i, g in enumerate(("i", "f", "g", "o"))}
        if has_b2:
            b2_sb = consts.tile([4, 128, H], F32)
            nc.sync.dma_start(out=b2_sb[:], in_=B2[:])
            b2_tiles = {g: b2_sb[i] for i, g in enumerate(("i", "f", "g", "o"))}
        bd_sb = None
        if has_bd:
            bd_sb = consts.tile([128, 1], F32)
            nc.sync.dma_start(out=bd_sb[:], in_=BD[:])

        c_fold = consts.tile([128, H], F32)  # persistent cell state

        def w1_rhs(off):
            return w1_sb[:, off:off + H]

        def u1_rhs(k):
            return lambda off: u1_sb[:, k, off:off + H]

        def w2_rhs(k):
            return lambda off: w2_sb[:, k, off:off + H]

        # ---- warmup: 64 steps of cell1 over the input sequence ----
        hT = None
        tail = None
        for t in range(T):
            x_term = (xt_sb[:, t, :], w1_rhs)
            if t == 0:
                zp = _emit_chains(nc, pools, [x_term], final=True)
                hT, tail = _emit_cell(nc, pools, zp, c_fold, True, idn_sb, b1_tiles)
                continue
            # pre-start next cell's x-side chains so the PE has work during
            # the previous cell's tail, then emit that tail
            zp = _emit_chains(nc, pools, [x_term], final=False)
            tail()
            h_terms = [(hT[:, k, :], u1_rhs(k)) for k in range(NK)]
            zp = _emit_chains(nc, pools, h_terms, zp=zp, final=True)
            hT, tail = _emit_cell(nc, pools, zp, c_fold, False, idn_sb, b1_tiles)
        tail()

        # ---- pred0 ----
        predT = _emit_pred(nc, pools, wd_sb, hT, OUT, 0, bd_sb)

        # ---- autoregressive: 31 steps of cell1 + cell2 ----
        for t in range(OUT_STEPS - 1):
            # predT is used as the LAST term so the pred chain of this step
            # has slack to finish while the h-terms stream
            terms1 = [(hT[:, k, :], u1_rhs(k)) for k in range(NK)] + [(predT, w1_rhs)]
            zp = _emit_chains(nc, pools, terms1, final=True)
            h1T, tail1 = _emit_cell(nc, pools, zp, c_fold, False, idn_sb, b1_tiles)
            tail1()
            terms2 = [(h1T[:, k, :], w2_rhs(k)) for k in range(NK)]
            zp = _emit_chains(nc, pools, terms2, final=True)
            hT, tail2 = _emit_cell(nc, pools, zp, c_fold, False, idn_sb, b2_tiles)
            tail2()
            predT = _emit_pred(nc, pools, wd_sb, hT, OUT, t + 1, bd_sb)

    nc.compile()
    return nc


def _fold_bias(b):
    """[4096] gate bias -> [4, 128, 512] folded tiles in (i,f,g,o) order."""
    out = np.zeros((4, 128, H), np.float32)
    for gi, gname in enumerate(("i", "f", "g", "o")):
        off = GATE_OFF[gname]
        out[gi, 0:64, :] = b[off:off + H][None, :]
        out[gi, 64:128, :] = b[off + H:off + 2 * H][None, :]
    return out


def kernel(inputs, mean, var, W1, U1, b1, W2, U2, b2, Wd, bd):
    x = np.asarray(inputs, np.float32)
    mean = np.asarray(mean, np.float32)
    var = np.asarray(var, np.float32)
    inv = 1.0 / np.sqrt(var + EPS)
    xn = ((x - mean) * inv - mean) * inv  # reference normalizes twice

    W1 = np.asarray(W1, np.float32)
    U1k = np.asarray(U1, np.float32).reshape(NK, 128, G).transpose(1, 0, 2)
    W2U2 = (np.asarray(W2, np.float32) + np.asarray(U2, np.float32))
    W2k = W2U2.reshape(NK, 128, G).transpose(1, 0, 2)
    WDk = np.asarray(Wd, np.float32).reshape(NK, 128, F).transpose(1, 0, 2)
    idn2 = np.zeros((128, 64), np.float32)
    idn2[0:64] = np.eye(64)
    idn2[64:128] = np.eye(64)

    b1 = np.asarray(b1, np.float32)
    b2 = np.asarray(b2, np.float32)
    bd = np.asarray(bd, np.float32)
    has_b1 = bool(np.any(b1))
    has_b2 = bool(np.any(b2))
    has_bd = bool(np.any(bd))

    key = (has_b1, has_b2, has_bd)
    if key not in _BUILD_CACHE:
        _BUILD_CACHE[key] = _build(*key)
    nc = _BUILD_CACHE[key]

    bf = ml_dtypes.bfloat16
    shared = {
        "u1": np.ascontiguousarray(U1k).astype(bf),
        "w1": W1.astype(bf),
        "w2": np.ascontiguousarray(W2k).astype(bf),
        "wd": np.ascontiguousarray(WDk).astype(bf),
        "idn": idn2.astype(bf),
    }
    if has_b1:
        shared["b1f"] = _fold_bias(b1)
    if has_b2:
        shared["b2f"] = _fold_bias(b2)
    if has_bd:
        shared["bdf"] = bd.reshape(128, 1).astype(np.float32)

    in_maps = []
    for c in range(NCORES):
        shard = xn[c * BS:(c + 1) * BS]              # [64, 64, 128]
        xt = np.ascontiguousarray(shard.transpose(2, 1, 0)).astype(bf)
        m = dict(shared)
        m["xt"] = xt
        in_maps.append(m)

    res = run_bass_kernel_spmd(nc, in_maps, core_ids=list(range(NCORES)))
    kernel.last_results = res

    # per-core out: [32, 128 feat, 64 batch] -> [64, 32, 128]
    parts = [res.results[c]["out"].transpose(2, 0, 1) for c in range(NCORES)]
    return np.ascontiguousarray(np.concatenate(parts, axis=0), dtype=np.float32)


# revision 15
# speedup vs baseline: 1.2423x; 1.1343x over previous
"""Trainium2 Bass kernel for the AutoRegressiveLSTM problem.

Strategy: data-parallel over batch (512 -> 64 rows per NeuronCore, 8 cores,
zero inter-core communication). All weights resident in SBUF as bf16;
matmuls in bf16 with fp32 PSUM accumulation (validated ~4e-3 max rel err
through the full 95-step recurrence).

Per-core layouts:
  - LSTM state h is kept TRANSPOSED (hT, [unit, batch]) because the
    TensorEngine computes out = lhsT.T @ rhs: z[batch, gates] needs
    stationary hT k-tiles [128 units, 64 batch].
  - Gate pre-activations z land in PSUM "gate-folded": each [128, 512]
    PSUM tile holds one gate, partitions 0:64 = units 0:512 (batch-major),
    partitions 64:128 = units 512:1024. The two halves are two independent
    matmul accumulation chains targeting different PE column groups, which
    the hardware runs concurrently (recovers full 128-wide array
    utilization despite the 64-row batch shard).
  - c / h state stays in the same folded [128, 512] layout, so all
    elementwise ops run at full 128-partition width.
  - h is un-folded back to hT via 8 PE transposes per cell.
  - pred (the Dense output) is computed transposed (predT = Wd.T @ hT) and
    written per-step to DRAM as [128 feat, 64 batch]; the host unshards.

The double normalization of the input and W2+U2 (cell2 sees x == h) are
folded on the host.
"""

from contextlib import ExitStack

import numpy as np
import ml_dtypes

import concourse.bass as bass
import concourse.tile as tile
from concourse import bacc, mybir
from concourse.bass_utils import run_bass_kernel_spmd

BF16 = mybir.dt.bfloat16
F32 = mybir.dt.float32
AF = mybir.ActivationFunctionType

NCORES = 8
B_FULL = 512
BS = B_FULL // NCORES   # 64 batch rows per core
T = 64                  # warmup sequence length
F = 128                 # features
U = 1024                # LSTM units
G = 4 * U               # 4096 gate columns
NK = U // 128           # 8 contraction k-tiles
OUT_STEPS = 32
EPS = 1e-7

# gate column ranges in the natural [i f g o] weight layout, split in two
# 512-wide halves; each pair (lo, hi) is one gate's two halves and maps to
# PSUM partitions [0:64] / [64:128] of one [128, 512] tile.
GATE_OFF = {"i": 0, "f": U, "g": 2 * U, "o": 3 * U}
H = 512  # half-gate width

_BUILD_CACHE = {}


def _emit_chains(nc, pools, terms, zp=None, final=True):
    """Emit the paired matmul accumulation chains for all four gates.

    terms: list of (stationary AP, rhs_fn(col_off) -> moving AP), appended
    to the (possibly pre-started) accumulation groups in `zp`.
    Returns the zp dict {gate: psum tile [128, 512]}.
    """
    psum = pools["psum"]
    new = zp is None
    if new:
        zp = {g: psum.tile([128, H], F32, name=f"z_{g}", tag="zp") for g in ("g", "i", "f", "o")}
    nt = len(terms)
    for gate in ("g", "i", "f", "o"):
        off = GATE_OFF[gate]
        z = zp[gate]
        for j, (stat, rf) in enumerate(terms):
            start = new and j == 0
            stop = final and j == nt - 1
            # the two chains write disjoint partition halves of one bank;
            # the sim's zero-region group check is partition-blind, so skip
            nc.tensor.matmul(
                z[0:64, :], stat, rf(off),
                start=start, stop=stop, skip_group_check=True,
            )
            nc.tensor.matmul(
                z[64:128, :], stat, rf(off + H),
                start=start, stop=stop, skip_group_check=True,
            )
    return zp


NSL = 2            # tail slices
SW = H // NSL      # 256 cols per slice
# hT k-tiles are produced slice-by-slice: slice s yields k in {2s, 2s+1}
# (bank A, units chunk 0) and {4+2s, 4+2s+1} (bank B, chunk 1).
K_PROD_ORDER = [0, 1, 4, 5, 2, 3, 6, 7]


def _emit_cell(nc, pools, zp, c_fold, first, idn2, bias_tile=None):
    """Activations + state update for one LSTM cell whose gate matmuls are
    already emitted into `zp`. Returns (hT tile, tail_fn(interleave_fn)).

    Everything that can run during the o-gate matmul stream (g/i/f
    activations, c update, tanh(c)) is emitted inline. tail_fn emits the
    o-dependent tail (sigmoid(o) -> h -> transposes -> hT copies) in NSL
    pipelined slices, plus anti-HAM dummy transposes so the PE clock-gate
    stays warm through the tail. interleave_fn(s) is called after each
    slice's copies to let the caller drop PE work (the pred matmuls) into
    the tail's idle windows.
    """
    gates, temps = pools["gates"], pools["temps"]
    if bias_tile is not None:
        for gate in ("g", "i", "f", "o"):
            nc.vector.tensor_add(zp[gate][:, :], zp[gate][:, :], bias_tile[gate])
    acts = {}
    for gate in ("g", "i", "f"):
        a = gates.tile([128, H], F32, tag="gact")
        nc.scalar.activation(a, zp[gate][:, :], AF.Tanh if gate == "g" else AF.Sigmoid)
        acts[gate] = a

    # c = f*c + i*g ; wide muls, per-slice adds so tanh(c) slices can start
    ig = temps.tile([128, H], F32, tag="tmp")
    nc.vector.tensor_mul(ig, acts["i"], acts["g"])
    if not first:
        fc = temps.tile([128, H], F32, tag="tmp")
        nc.vector.tensor_mul(fc, acts["f"], c_fold[:, :])
    tc_s = []
    for s in range(NSL):
        sl = slice(SW * s, SW * (s + 1))
        if first:
            nc.vector.tensor_copy(c_fold[:, sl], ig[:, sl])
        else:
            nc.vector.tensor_add(c_fold[:, sl], fc[:, sl], ig[:, sl])
        t = gates.tile([128, SW], F32, tag="tcs")
        nc.scalar.activation(t, c_fold[:, sl], AF.Tanh)
        tc_s.append(t)

    h_fold = pools["hfold"].tile([128, H], BF16, tag="hfold")
    hT = pools["hT"].tile([128, NK, BS], BF16, tag="hT")

    def tail(interleave_fn=None):
        hpsA = pools["psum_hA"].tile([128, NSL * 2 + 1, BS], BF16, tag="hpsA")
        hpsB = pools["psum_hB"].tile([128, NSL * 2, BS], BF16, tag="hpsB")
        for s in range(NSL):
            sl = slice(SW * s, SW * (s + 1))
            o_t = gates.tile([128, SW], F32, tag="tcs")
            nc.scalar.activation(o_t, zp["o"][:, sl], AF.Sigmoid)
            nc.vector.tensor_mul(h_fold[:, sl], o_t, tc_s[s])
            # full-width transposes; each PSUM bank sees only one row-group
            for kk in (2 * s, 2 * s + 1):
                blk = slice(128 * kk, 128 * (kk + 1))
                nc.tensor.transpose(hpsA[:, kk, :], h_fold[0:64, blk], idn2[0:64, :])
                nc.tensor.transpose(hpsB[:, kk, :], h_fold[64:128, blk], idn2[64:128, :])
            nc.vector.tensor_copy(hT[:, 2 * s:2 * s + 2, :], hpsA[:, 2 * s:2 * s + 2, :])
            nc.vector.tensor_copy(hT[:, 4 + 2 * s:6 + 2 * s, :], hpsB[:, 2 * s:2 * s + 2, :])
            if interleave_fn is not None:
                interleave_fn(s)

    return hT, tail


def _emit_pred_chain(nc, pools, wd_sb, hT):
    """Emit the 8 pred matmuls in hT production order; returns psum tile.
    Designed to be dropped into a cell tail via its interleave hook."""
    pd = pools["psum_p"].tile([128, BS], F32, tag="pd")

    def emit_slice(s):
        ks = [2 * s, 2 * s + 1, 4 + 2 * s, 5 + 2 * s]
        for j, k in enumerate(ks):
            nc.tensor.matmul(
                pd[:, :], wd_sb[:, k, :], hT[:, k, :],
                start=(s == 0 and j == 0), stop=(s == NSL - 1 and j == 3),
            )
    return pd, emit_slice


def _emit_pred_finish(nc, pools, pd, out_dram, step_idx, bd_sb=None):
    pdv = pools["temps"].tile([128, BS], F32, tag="pdv")
    if bd_sb is not None:
        nc.vector.tensor_scalar_add(pdv, pd[:, :], bd_sb[:, 0:1])
    else:
        nc.vector.tensor_copy(pdv[:, :], pd[:, :])
    nc.sync.dma_start(out=out_dram[step_idx], in_=pdv[:, :])
    predT = pools["predT"].tile([128, BS], BF16, tag="predT")
    nc.vector.tensor_copy(predT[:, :], pdv[:, :])
    return predT


def _build(has_b1, has_b2, has_bd):
    nc = bacc.Bacc("TRN2", target_bir_lowering=False, debug=False)

    XT = nc.declare_dram_parameter("xt", [F, T, BS], BF16, isOutput=False)
    U1 = nc.declare_dram_parameter("u1", [128, NK, G], BF16, isOutput=False)
    W1 = nc.declare_dram_parameter("w1", [F, G], BF16, isOutput=False)
    W2 = nc.declare_dram_parameter("w2", [128, NK, G], BF16, isOutput=False)
    WD = nc.declare_dram_parameter("wd", [128, NK, F], BF16, isOutput=False)
    IDN = nc.declare_dram_parameter("idn", [128, 64], BF16, isOutput=False)
    if has_b1:
        B1 = nc.declare_dram_parameter("b1f", [4, 128, H], F32, isOutput=False)
    if has_b2:
        B2 = nc.declare_dram_parameter("b2f", [4, 128, H], F32, isOutput=False)
    if has_bd:
        BD = nc.declare_dram_parameter("bdf", [128, 1], F32, isOutput=False)
    OUT = nc.declare_dram_parameter("out", [OUT_STEPS, F, BS], F32, isOutput=True)

    with tile.TileContext(nc) as tc, ExitStack() as ctx:
        consts = ctx.enter_context(tc.tile_pool(name="consts", bufs=1))
        pools = {
            "psum": ctx.enter_context(tc.tile_pool(name="psum", bufs=5, space="PSUM")),
            "psum_hA": ctx.enter_context(tc.tile_pool(name="psum_hA", bufs=1, space="PSUM")),
            "psum_hB": ctx.enter_context(tc.tile_pool(name="psum_hB", bufs=1, space="PSUM")),
            "psum_p": ctx.enter_context(tc.tile_pool(name="psum_p", bufs=1, space="PSUM")),
            "gates": ctx.enter_context(tc.tile_pool(name="gates", bufs=8)),
            "temps": ctx.enter_context(tc.tile_pool(name="temps", bufs=4)),
            "hfold": ctx.enter_context(tc.tile_pool(name="hfold", bufs=2)),
            "hT": ctx.enter_context(tc.tile_pool(name="hT", bufs=2)),
            "predT": ctx.enter_context(tc.tile_pool(name="predT", bufs=2)),
        }

        xt_sb = consts.tile([F, T, BS], BF16)
        u1_sb = consts.tile([128, NK, G], BF16)
        w1_sb = consts.tile([F, G], BF16)
        w2_sb = consts.tile([128, NK, G], BF16)
        wd_sb = consts.tile([128, NK, F], BF16)
        idn_sb = consts.tile([128, 64], BF16)
        nc.sync.dma_start(out=xt_sb[:], in_=XT[:])
        nc.sync.dma_start(out=w1_sb[:], in_=W1[:])
        for k in range(NK):  # split big weight DMAs so early steps start sooner
            nc.sync.dma_start(out=u1_sb[:, k, :], in_=U1[:, k, :])
        for k in range(NK):
            nc.sync.dma_start(out=w2_sb[:, k, :], in_=W2[:, k, :])
        nc.sync.dma_start(out=wd_sb[:], in_=WD[:])
        nc.sync.dma_start(out=idn_sb[:], in_=IDN[:])

        b1_tiles = b2_tiles = None
        if has_b1:
            b1_sb = consts.tile([4, 128, H], F32)
            nc.sync.dma_start(out=b1_sb[:], in_=B1[:])
            b1_tiles = {g: b1_sb[i] for i, g in enumerate(("i", "f", "g", "o"))}
        if has_b2:
            b2_sb = consts.tile([4, 128, H], F32)
            nc.sync.dma_start(out=b2_sb[:], in_=B2[:])
            b2_tiles = {g: b2_sb[i] for i, g in enumerate(("i", "f", "g", "o"))}
        bd_sb = None
        if has_bd:
            bd_sb = consts.tile([128, 1], F32)
            nc.sync.dma_start(out=bd_sb[:], in_=BD[:])

        c_fold = consts.tile([128, H], F32)  # persistent cell state

        def w1_rhs(off):
            return w1_sb[:, off:off + H]

        def u1_rhs(k):
            return lambda off: u1_sb[:, k, off:off + H]

        def w2_rhs(k):
            return lambda off: w2_sb[:, k, off:off + H]

        # ---- warmup: 64 steps of cell1 over the input sequence ----
        hT = None
        tail = None
        for t in range(T):
            x_term = (xt_sb[:, t, :], w1_rhs)
            if t == 0:
                zp = _emit_chains(nc, pools, [x_term], final=True)
                hT, tail = _emit_cell(nc, pools, zp, c_fold, True, idn_sb, b1_tiles)
                continue
            # pre-start next cell's x-side chains so the PE has work during
            # the previous cell's tail, then emit that tail
            zp = _emit_chains(nc, pools, [x_term], final=False)
            tail()
            h_terms = [(hT[:, k, :], u1_rhs(k)) for k in K_PROD_ORDER]
            zp = _emit_chains(nc, pools, h_terms, zp=zp, final=True)
            hT, tail = _emit_cell(nc, pools, zp, c_fold, False, idn_sb, b1_tiles)
        # ---- pred0 interleaved into the last warmup tail ----
        pd, pred_slice = _emit_pred_chain(nc, pools, wd_sb, hT)
        tail(pred_slice)
        predT = _emit_pred_finish(nc, pools, pd, OUT, 0, bd_sb)

        # ---- autoregressive: 31 steps of cell1 + cell2 ----
        for t in range(OUT_STEPS - 1):
            # predT is the LAST term so its producer chain (in the previous
            # tail) has slack while the h-terms stream
            terms1 = [(hT[:, k, :], u1_rhs(k)) for k in K_PROD_ORDER] + [(predT, w1_rhs)]
            zp = _emit_chains(nc, pools, terms1, final=True)
            h1T, tail1 = _emit_cell(nc, pools, zp, c_fold, False, idn_sb, b1_tiles)
            tail1()
            terms2 = [(h1T[:, k, :], w2_rhs(k)) for k in K_PROD_ORDER]
            zp = _emit_chains(nc, pools, terms2, final=True)
            hT, tail2 = _emit_cell(nc, pools, zp, c_fold, False, idn_sb, b2_tiles)
            pd, pred_slice = _emit_pred_chain(nc, pools, wd_sb, hT)
            tail2(pred_slice)
            predT = _emit_pred_finish(nc, pools, pd, OUT, t + 1, bd_sb)

    nc.compile()
    return nc


def _fold_bias(b):
    """[4096] gate bias -> [4, 128, 512] folded tiles in (i,f,g,o) order."""
    out = np.zeros((4, 128, H), np.float32)
    for gi, gname in enumerate(("i", "f", "g", "o")):
        off = GATE_OFF[gname]
        out[gi, 0:64, :] = b[off:off + H][None, :]
        out[gi, 64:128, :] = b[off + H:off + 2 * H][None, :]
    return out


def kernel(inputs, mean, var, W1, U1, b1, W2, U2, b2, Wd, bd):
    x = np.asarray(inputs, np.float32)
    mean = np.asarray(mean, np.float32)
    var = np.asarray(var, np.float32)
    inv = 1.0 / np.sqrt(var + EPS)
    xn = ((x - mean) * inv - mean) * inv  # reference normalizes twice

    W1 = np.asarray(W1, np.float32)
    U1k = np.asarray(U1, np.float32).reshape(NK, 128, G).transpose(1, 0, 2)
    W2U2 = (np.asarray(W2, np.float32) + np.asarray(U2, np.float32))
    W2k = W2U2.reshape(NK, 128, G).transpose(1, 0, 2)
    WDk = np.asarray(Wd, np.float32).reshape(NK, 128, F).transpose(1, 0, 2)
    idn2 = np.zeros((128, 64), np.float32)
    idn2[0:64] = np.eye(64)
    idn2[64:128] = np.eye(64)

    b1 = np.asarray(b1, np.float32)
    b2 = np.asarray(b2, np.float32)
    bd = np.asarray(bd, np.float32)
    has_b1 = bool(np.any(b1))
    has_b2 = bool(np.any(b2))
    has_bd = bool(np.any(bd))

    key = (has_b1, has_b2, has_bd)
    if key not in _BUILD_CACHE:
        _BUILD_CACHE[key] = _build(*key)
    nc = _BUILD_CACHE[key]

    bf = ml_dtypes.bfloat16
    shared = {
        "u1": np.ascontiguousarray(U1k).astype(bf),
        "w1": W1.astype(bf),
        "w2": np.ascontiguousarray(W2k).astype(bf),
        "wd": np.ascontiguousarray(WDk).astype(bf),
        "idn": idn2.astype(bf),
    }
    if has_b1:
        shared["b1f"] = _fold_bias(b1)
    if has_b2:
        shared["b2f"] = _fold_bias(b2)
    if has_bd:
        shared["bdf"] = bd.reshape(128, 1).astype(np.float32)

    in_maps = []
    for c in range(NCORES):
        shard = xn[c * BS:(c + 1) * BS]              # [64, 64, 128]
        xt = np.ascontiguousarray(shard.transpose(2, 1, 0)).astype(bf)
        m = dict(shared)
        m["xt"] = xt
        in_maps.append(m)

    res = run_bass_kernel_spmd(nc, in_maps, core_ids=list(range(NCORES)))
    kernel.last_results = res

    # per-core out: [32, 128 feat, 64 batch] -> [64, 32, 128]
    parts = [res.results[c]["out"].transpose(2, 0, 1) for c in range(NCORES)]
    return np.ascontiguousarray(np.concatenate(parts, axis=0), dtype=np.float32)


# revision 16
# speedup vs baseline: 1.3128x; 1.0567x over previous
"""Trainium2 Bass kernel for the AutoRegressiveLSTM problem.

Strategy: data-parallel over batch (512 -> 64 rows per NeuronCore, 8 cores,
zero inter-core communication). All weights resident in SBUF as bf16;
matmuls in bf16 with fp32 PSUM accumulation (validated ~4e-3 max rel err
through the full 95-step recurrence).

Per-core layouts:
  - LSTM state h is kept TRANSPOSED (hT, [unit, batch]) because the
    TensorEngine computes out = lhsT.T @ rhs: z[batch, gates] needs
    stationary hT k-tiles [128 units, 64 batch].
  - Gate pre-activations z land in PSUM "gate-folded": each [128, 512]
    PSUM tile holds one gate, partitions 0:64 = units 0:512 (batch-major),
    partitions 64:128 = units 512:1024. The two halves are two independent
    matmul accumulation chains targeting different PE column groups, which
    the hardware runs concurrently (recovers full 128-wide array
    utilization despite the 64-row batch shard).
  - c / h state stays in the same folded [128, 512] layout, so all
    elementwise ops run at full 128-partition width.
  - h is un-folded back to hT via 8 PE transposes per cell.
  - pred (the Dense output) is computed transposed (predT = Wd.T @ hT) and
    written per-step to DRAM as [128 feat, 64 batch]; the host unshards.

The double normalization of the input and W2+U2 (cell2 sees x == h) are
folded on the host.
"""

from contextlib import ExitStack

import numpy as np
import ml_dtypes

import concourse.bass as bass
import concourse.tile as tile
from concourse import bacc, mybir
from concourse.bass_utils import run_bass_kernel_spmd

BF16 = mybir.dt.bfloat16
F32 = mybir.dt.float32
AF = mybir.ActivationFunctionType

NCORES = 8
B_FULL = 512
BS = B_FULL // NCORES   # 64 batch rows per core
T = 64                  # warmup sequence length
F = 128                 # features
U = 1024                # LSTM units
G = 4 * U               # 4096 gate columns
NK = U // 128           # 8 contraction k-tiles
OUT_STEPS = 32
EPS = 1e-7

# gate column ranges in the natural [i f g o] weight layout, split in two
# 512-wide halves; each pair (lo, hi) is one gate's two halves and maps to
# PSUM partitions [0:64] / [64:128] of one [128, 512] tile.
GATE_OFF = {"i": 0, "f": U, "g": 2 * U, "o": 3 * U}
H = 512  # half-gate width

_BUILD_CACHE = {}


def _emit_chains(nc, pools, terms, zp=None, final=True):
    """Emit the paired matmul accumulation chains for all four gates.

    terms: list of (stationary AP, rhs_fn(col_off) -> moving AP), appended
    to the (possibly pre-started) accumulation groups in `zp`.
    Returns the zp dict {gate: psum tile [128, 512]}.
    """
    psum = pools["psum"]
    new = zp is None
    if new:
        zp = {g: psum.tile([128, H], F32, name=f"z_{g}", tag="zp") for g in ("g", "i", "f", "o")}
    nt = len(terms)
    for gate in ("g", "i", "f", "o"):
        off = GATE_OFF[gate]
        z = zp[gate]
        for j, (stat, rf) in enumerate(terms):
            start = new and j == 0
            stop = final and j == nt - 1
            # the two chains write disjoint partition halves of one bank;
            # the sim's zero-region group check is partition-blind, so skip
            nc.tensor.matmul(
                z[0:64, :], stat, rf(off),
                start=start, stop=stop, skip_group_check=True,
            )
            nc.tensor.matmul(
                z[64:128, :], stat, rf(off + H),
                start=start, stop=stop, skip_group_check=True,
            )
    return zp


NSL = 2            # tail slices
SW = H // NSL      # 256 cols per slice
# hT k-tiles are produced slice-by-slice: slice s yields k in {2s, 2s+1}
# (bank A, units chunk 0) and {4+2s, 4+2s+1} (bank B, chunk 1).
K_PROD_ORDER = [0, 1, 4, 5, 2, 3, 6, 7]


def _emit_cell(nc, pools, zp, c_fold, first, idn2, bias_tile=None):
    """Activations + state update for one LSTM cell whose gate matmuls are
    already emitted into `zp`. Returns (hT tile, tail_fn(interleave_fn)).

    Everything that can run during the o-gate matmul stream (g/i/f
    activations, c update, tanh(c)) is emitted inline. tail_fn emits the
    o-dependent tail (sigmoid(o) -> h -> transposes -> hT copies) in NSL
    pipelined slices, plus anti-HAM dummy transposes so the PE clock-gate
    stays warm through the tail. interleave_fn(s) is called after each
    slice's copies to let the caller drop PE work (the pred matmuls) into
    the tail's idle windows.
    """
    gates, temps = pools["gates"], pools["temps"]
    if bias_tile is not None:
        for gate in ("g", "i", "f", "o"):
            nc.vector.tensor_add(zp[gate][:, :], zp[gate][:, :], bias_tile[gate])
    acts = {}
    for gate in ("g", "i", "f"):
        a = gates.tile([128, H], F32, tag="gact")
        nc.scalar.activation(a, zp[gate][:, :], AF.Tanh if gate == "g" else AF.Sigmoid)
        acts[gate] = a

    # c = f*c + i*g ; wide muls, per-slice adds so tanh(c) slices can start
    ig = temps.tile([128, H], F32, tag="tmp")
    nc.vector.tensor_mul(ig, acts["i"], acts["g"])
    if not first:
        fc = temps.tile([128, H], F32, tag="tmp")
        nc.vector.tensor_mul(fc, acts["f"], c_fold[:, :])
    for s in range(NSL):
        sl = slice(SW * s, SW * (s + 1))
        if first:
            nc.vector.tensor_copy(c_fold[:, sl], ig[:, sl])
        else:
            nc.vector.tensor_add(c_fold[:, sl], fc[:, sl], ig[:, sl])

    h_fold = pools["hfold"].tile([128, H], BF16, tag="hfold")
    hT = pools["hT"].tile([128, NK, BS], BF16, tag="hT")

    def tail(interleave_fn=None):
        hpsA = pools["psum_hA"].tile([128, NSL * 2 + 1, BS], BF16, tag="hpsA")
        hpsB = pools["psum_hB"].tile([128, NSL * 2, BS], BF16, tag="hpsB")
        for s in range(NSL):
            sl = slice(SW * s, SW * (s + 1))
            tc_t = gates.tile([128, SW], F32, tag="tcs")
            nc.scalar.activation(tc_t, c_fold[:, sl], AF.Tanh)
            o_t = gates.tile([128, SW], F32, tag="tcs")
            nc.scalar.activation(o_t, zp["o"][:, sl], AF.Sigmoid)
            nc.vector.tensor_mul(h_fold[:, sl], o_t, tc_t)
            # full-width transposes; each PSUM bank sees only one row-group
            for kk in (2 * s, 2 * s + 1):
                blk = slice(128 * kk, 128 * (kk + 1))
                nc.tensor.transpose(hpsA[:, kk, :], h_fold[0:64, blk], idn2[0:64, :])
                nc.tensor.transpose(hpsB[:, kk, :], h_fold[64:128, blk], idn2[64:128, :])
            nc.vector.tensor_copy(hT[:, 2 * s:2 * s + 2, :], hpsA[:, 2 * s:2 * s + 2, :])
            nc.vector.tensor_copy(hT[:, 4 + 2 * s:6 + 2 * s, :], hpsB[:, 2 * s:2 * s + 2, :])
            if interleave_fn is not None:
                interleave_fn(s)

    return hT, tail


def _emit_pred_chain(nc, pools, wd_sb, hT):
    """Emit the 8 pred matmuls in hT production order; returns psum tile.
    Designed to be dropped into a cell tail via its interleave hook."""
    pd = pools["psum_p"].tile([128, BS], F32, tag="pd")

    def emit_slice(s):
        ks = [2 * s, 2 * s + 1, 4 + 2 * s, 5 + 2 * s]
        for j, k in enumerate(ks):
            nc.tensor.matmul(
                pd[:, :], wd_sb[:, k, :], hT[:, k, :],
                start=(s == 0 and j == 0), stop=(s == NSL - 1 and j == 3),
            )
    return pd, emit_slice


def _emit_pred_finish(nc, pools, pd, out_dram, step_idx, bd_sb=None):
    pdv = pools["temps"].tile([128, BS], F32, tag="pdv")
    if bd_sb is not None:
        nc.vector.tensor_scalar_add(pdv, pd[:, :], bd_sb[:, 0:1])
    else:
        nc.vector.tensor_copy(pdv[:, :], pd[:, :])
    nc.sync.dma_start(out=out_dram[step_idx], in_=pdv[:, :])
    predT = pools["predT"].tile([128, BS], BF16, tag="predT")
    nc.vector.tensor_copy(predT[:, :], pdv[:, :])
    return predT


def _build(has_b1, has_b2, has_bd):
    nc = bacc.Bacc("TRN2", target_bir_lowering=False, debug=False)

    XT = nc.declare_dram_parameter("xt", [F, T, BS], BF16, isOutput=False)
    U1 = nc.declare_dram_parameter("u1", [128, NK, G], BF16, isOutput=False)
    W1 = nc.declare_dram_parameter("w1", [F, G], BF16, isOutput=False)
    W2 = nc.declare_dram_parameter("w2", [128, NK, G], BF16, isOutput=False)
    WD = nc.declare_dram_parameter("wd", [128, NK, F], BF16, isOutput=False)
    IDN = nc.declare_dram_parameter("idn", [128, 64], BF16, isOutput=False)
    if has_b1:
        B1 = nc.declare_dram_parameter("b1f", [4, 128, H], F32, isOutput=False)
    if has_b2:
        B2 = nc.declare_dram_parameter("b2f", [4, 128, H], F32, isOutput=False)
    if has_bd:
        BD = nc.declare_dram_parameter("bdf", [128, 1], F32, isOutput=False)
    OUT = nc.declare_dram_parameter("out", [OUT_STEPS, F, BS], F32, isOutput=True)

    with tile.TileContext(nc) as tc, ExitStack() as ctx:
        consts = ctx.enter_context(tc.tile_pool(name="consts", bufs=1))
        pools = {
            "psum": ctx.enter_context(tc.tile_pool(name="psum", bufs=5, space="PSUM")),
            "psum_hA": ctx.enter_context(tc.tile_pool(name="psum_hA", bufs=1, space="PSUM")),
            "psum_hB": ctx.enter_context(tc.tile_pool(name="psum_hB", bufs=1, space="PSUM")),
            "psum_p": ctx.enter_context(tc.tile_pool(name="psum_p", bufs=1, space="PSUM")),
            "gates": ctx.enter_context(tc.tile_pool(name="gates", bufs=8)),
            "temps": ctx.enter_context(tc.tile_pool(name="temps", bufs=4)),
            "hfold": ctx.enter_context(tc.tile_pool(name="hfold", bufs=2)),
            "hT": ctx.enter_context(tc.tile_pool(name="hT", bufs=2)),
            "predT": ctx.enter_context(tc.tile_pool(name="predT", bufs=2)),
        }

        xt_sb = consts.tile([F, T, BS], BF16)
        u1_sb = consts.tile([128, NK, G], BF16)
        w1_sb = consts.tile([F, G], BF16)
        w2_sb = consts.tile([128, NK, G], BF16)
        wd_sb = consts.tile([128, NK, F], BF16)
        idn_sb = consts.tile([128, 64], BF16)
        nc.sync.dma_start(out=xt_sb[:], in_=XT[:])
        nc.sync.dma_start(out=w1_sb[:], in_=W1[:])
        for k in range(NK):  # split big weight DMAs so early steps start sooner
            nc.sync.dma_start(out=u1_sb[:, k, :], in_=U1[:, k, :])
        for k in range(NK):
            nc.sync.dma_start(out=w2_sb[:, k, :], in_=W2[:, k, :])
        nc.sync.dma_start(out=wd_sb[:], in_=WD[:])
        nc.sync.dma_start(out=idn_sb[:], in_=IDN[:])

        b1_tiles = b2_tiles = None
        if has_b1:
            b1_sb = consts.tile([4, 128, H], F32)
            nc.sync.dma_start(out=b1_sb[:], in_=B1[:])
            b1_tiles = {g: b1_sb[i] for i, g in enumerate(("i", "f", "g", "o"))}
        if has_b2:
            b2_sb = consts.tile([4, 128, H], F32)
            nc.sync.dma_start(out=b2_sb[:], in_=B2[:])
            b2_tiles = {g: b2_sb[i] for i, g in enumerate(("i", "f", "g", "o"))}
        bd_sb = None
        if has_bd:
            bd_sb = consts.tile([128, 1], F32)
            nc.sync.dma_start(out=bd_sb[:], in_=BD[:])

        c_fold = consts.tile([128, H], F32)  # persistent cell state

        def w1_rhs(off):
            return w1_sb[:, off:off + H]

        def u1_rhs(k):
            return lambda off: u1_sb[:, k, off:off + H]

        def w2_rhs(k):
            return lambda off: w2_sb[:, k, off:off + H]

        # ---- warmup: 64 steps of cell1 over the input sequence ----
        hT = None
        tail = None
        for t in range(T):
            x_term = (xt_sb[:, t, :], w1_rhs)
            if t == 0:
                zp = _emit_chains(nc, pools, [x_term], final=True)
                hT, tail = _emit_cell(nc, pools, zp, c_fold, True, idn_sb, b1_tiles)
                continue
            # pre-start next cell's x-side chains so the PE has work during
            # the previous cell's tail, then emit that tail
            zp = _emit_chains(nc, pools, [x_term], final=False)
            tail()
            h_terms = [(hT[:, k, :], u1_rhs(k)) for k in K_PROD_ORDER]
            zp = _emit_chains(nc, pools, h_terms, zp=zp, final=True)
            hT, tail = _emit_cell(nc, pools, zp, c_fold, False, idn_sb, b1_tiles)
        # ---- pred0 interleaved into the last warmup tail ----
        pd, pred_slice = _emit_pred_chain(nc, pools, wd_sb, hT)
        tail(pred_slice)
        predT = _emit_pred_finish(nc, pools, pd, OUT, 0, bd_sb)

        # ---- autoregressive: 31 steps of cell1 + cell2 ----
        for t in range(OUT_STEPS - 1):
            # predT is the LAST term so its producer chain (in the previous
            # tail) has slack while the h-terms stream
            terms1 = [(hT[:, k, :], u1_rhs(k)) for k in K_PROD_ORDER] + [(predT, w1_rhs)]
            zp = _emit_chains(nc, pools, terms1, final=True)
            h1T, tail1 = _emit_cell(nc, pools, zp, c_fold, False, idn_sb, b1_tiles)
            tail1()
            terms2 = [(h1T[:, k, :], w2_rhs(k)) for k in K_PROD_ORDER]
            zp = _emit_chains(nc, pools, terms2, final=True)
            hT, tail2 = _emit_cell(nc, pools, zp, c_fold, False, idn_sb, b2_tiles)
            pd, pred_slice = _emit_pred_chain(nc, pools, wd_sb, hT)
            tail2(pred_slice)
            predT = _emit_pred_finish(nc, pools, pd, OUT, t + 1, bd_sb)

    nc.compile()
    return nc


def _fold_bias(b):
    """[4096] gate bias -> [4, 128, 512] folded tiles in (i,f,g,o) order."""
    out = np.zeros((4, 128, H), np.float32)
    for gi, gname in enumerate(("i", "f", "g", "o")):
        off = GATE_OFF[gname]
        out[gi, 0:64, :] = b[off:off + H][None, :]
        out[gi, 64:128, :] = b[off + H:off + 2 * H][None, :]
    return out


def kernel(inputs, mean, var, W1, U1, b1, W2, U2, b2, Wd, bd):
    x = np.asarray(inputs, np.float32)
    mean = np.asarray(mean, np.float32)
    var = np.asarray(var, np.float32)
    inv = 1.0 / np.sqrt(var + EPS)
    xn = ((x - mean) * inv - mean) * inv  # reference normalizes twice

    W1 = np.asarray(W1, np.float32)
    U1k = np.asarray(U1, np.float32).reshape(NK, 128, G).transpose(1, 0, 2)
    W2U2 = (np.asarray(W2, np.float32) + np.asarray(U2, np.float32))
    W2k = W2U2.reshape(NK, 128, G).transpose(1, 0, 2)
    WDk = np.asarray(Wd, np.float32).reshape(NK, 128, F).transpose(1, 0, 2)
    idn2 = np.zeros((128, 64), np.float32)
    idn2[0:64] = np.eye(64)
    idn2[64:128] = np.eye(64)

    b1 = np.asarray(b1, np.float32)
    b2 = np.asarray(b2, np.float32)
    bd = np.asarray(bd, np.float32)
    has_b1 = bool(np.any(b1))
    has_b2 = bool(np.any(b2))
    has_bd = bool(np.any(bd))

    key = (has_b1, has_b2, has_bd)
    if key not in _BUILD_CACHE:
        _BUILD_CACHE[key] = _build(*key)
    nc = _BUILD_CACHE[key]

    bf = ml_dtypes.bfloat16
    shared = {
        "u1": np.ascontiguousarray(U1k).astype(bf),
        "w1": W1.astype(bf),
        "w2": np.ascontiguousarray(W2k).astype(bf),
        "wd": np.ascontiguousarray(WDk).astype(bf),
        "idn": idn2.astype(bf),
    }
    if has_b1:
        shared["b1f"] = _fold_bias(b1)
    if has_b2:
        shared["b2f"] = _fold_bias(b2)
    if has_bd:
        shared["bdf"] = bd.reshape(128, 1).astype(np.float32)

    in_maps = []
    for c in range(NCORES):
        shard = xn[c * BS:(c + 1) * BS]              # [64, 64, 128]
        xt = np.ascontiguousarray(shard.transpose(2, 1, 0)).astype(bf)
        m = dict(shared)
        m["xt"] = xt
        in_maps.append(m)

    res = run_bass_kernel_spmd(nc, in_maps, core_ids=list(range(NCORES)))
    kernel.last_results = res

    # per-core out: [32, 128 feat, 64 batch] -> [64, 32, 128]
    parts = [res.results[c]["out"].transpose(2, 0, 1) for c in range(NCORES)]
    return np.ascontiguousarray(np.concatenate(parts, axis=0), dtype=np.float32)


# revision 17
# speedup vs baseline: 1.3162x; 1.0026x over previous
"""Trainium2 Bass kernel for the AutoRegressiveLSTM problem.

Strategy: data-parallel over batch (512 -> 64 rows per NeuronCore, 8 cores,
zero inter-core communication). All weights resident in SBUF as bf16;
matmuls in bf16 with fp32 PSUM accumulation (validated ~4e-3 max rel err
through the full 95-step recurrence).

Per-core layouts:
  - LSTM state h is kept TRANSPOSED (hT, [unit, batch]) because the
    TensorEngine computes out = lhsT.T @ rhs: z[batch, gates] needs
    stationary hT k-tiles [128 units, 64 batch].
  - Gate pre-activations z land in PSUM "gate-folded": each [128, 512]
    PSUM tile holds one gate, partitions 0:64 = units 0:512 (batch-major),
    partitions 64:128 = units 512:1024. The two halves are two independent
    matmul accumulation chains targeting different PE column groups, which
    the hardware runs concurrently (recovers full 128-wide array
    utilization despite the 64-row batch shard).
  - c / h state stays in the same folded [128, 512] layout, so all
    elementwise ops run at full 128-partition width.
  - h is un-folded back to hT via 8 PE transposes per cell.
  - pred (the Dense output) is computed transposed (predT = Wd.T @ hT) and
    written per-step to DRAM as [128 feat, 64 batch]; the host unshards.

The double normalization of the input and W2+U2 (cell2 sees x == h) are
folded on the host.
"""

from contextlib import ExitStack

import numpy as np
import ml_dtypes

import concourse.bass as bass
import concourse.tile as tile
from concourse import bacc, mybir
from concourse.bass_utils import run_bass_kernel_spmd

BF16 = mybir.dt.bfloat16
F32 = mybir.dt.float32
AF = mybir.ActivationFunctionType

NCORES = 8
B_FULL = 512
BS = B_FULL // NCORES   # 64 batch rows per core
T = 64                  # warmup sequence length
F = 128                 # features
U = 1024                # LSTM units
G = 4 * U               # 4096 gate columns
NK = U // 128           # 8 contraction k-tiles
OUT_STEPS = 32
EPS = 1e-7

# gate column ranges in the natural [i f g o] weight layout, split in two
# 512-wide halves; each pair (lo, hi) is one gate's two halves and maps to
# PSUM partitions [0:64] / [64:128] of one [128, 512] tile.
GATE_OFF = {"i": 0, "f": U, "g": 2 * U, "o": 3 * U}
H = 512  # half-gate width

_BUILD_CACHE = {}


def _emit_chains(nc, pools, terms, zp=None, final=True):
    """Emit the paired matmul accumulation chains for all four gates.

    terms: list of (stationary AP, rhs_fn(col_off) -> moving AP), appended
    to the (possibly pre-started) accumulation groups in `zp`.
    Returns the zp dict {gate: psum tile [128, 512]}.
    """
    psum = pools["psum"]
    new = zp is None
    if new:
        zp = {g: psum.tile([128, H], F32, name=f"z_{g}", tag="zp") for g in ("g", "i", "f", "o")}
    nt = len(terms)
    for gate in ("g", "i", "f", "o"):
        off = GATE_OFF[gate]
        z = zp[gate]
        for j, (stat, rf) in enumerate(terms):
            start = new and j == 0
            stop = final and j == nt - 1
            # the two chains write disjoint partition halves of one bank;
            # the sim's zero-region group check is partition-blind, so skip
            nc.tensor.matmul(
                z[0:64, :], stat, rf(off),
                start=start, stop=stop, skip_group_check=True,
            )
            nc.tensor.matmul(
                z[64:128, :], stat, rf(off + H),
                start=start, stop=stop, skip_group_check=True,
            )
    return zp


NSL = 2            # tail slices
SW = H // NSL      # 256 cols per slice
# hT k-tiles are produced slice-by-slice: slice s yields k in {2s, 2s+1}
# (bank A, units chunk 0) and {4+2s, 4+2s+1} (bank B, chunk 1).
K_PROD_ORDER = [0, 1, 4, 5, 2, 3, 6, 7]


def _emit_cell(nc, pools, zp, c_fold, first, idn2, bias_tile=None):
    """Activations + state update for one LSTM cell whose gate matmuls are
    already emitted into `zp`. Returns (hT tile, tail_fn(interleave_fn)).

    Everything that can run during the o-gate matmul stream (g/i/f
    activations, c update, tanh(c)) is emitted inline. tail_fn emits the
    o-dependent tail (sigmoid(o) -> h -> transposes -> hT copies) in NSL
    pipelined slices, plus anti-HAM dummy transposes so the PE clock-gate
    stays warm through the tail. interleave_fn(s) is called after each
    slice's copies to let the caller drop PE work (the pred matmuls) into
    the tail's idle windows.
    """
    gates, temps = pools["gates"], pools["temps"]
    if bias_tile is not None:
        for gate in ("g", "i", "f", "o"):
            nc.vector.tensor_add(zp[gate][:, :], zp[gate][:, :], bias_tile[gate])
    acts = {}
    for gate in ("g", "i", "f"):
        a = gates.tile([128, H], F32, tag="gact")
        nc.scalar.activation(a, zp[gate][:, :], AF.Tanh if gate == "g" else AF.Sigmoid)
        acts[gate] = a

    # c = f*c + i*g ; wide muls, per-slice adds so tanh(c) slices can start
    ig = temps.tile([128, H], F32, tag="tmp")
    nc.vector.tensor_mul(ig, acts["i"], acts["g"])
    if not first:
        fc = temps.tile([128, H], F32, tag="tmp")
        nc.vector.tensor_mul(fc, acts["f"], c_fold[:, :])
    for s in range(NSL):
        sl = slice(SW * s, SW * (s + 1))
        if first:
            nc.vector.tensor_copy(c_fold[:, sl], ig[:, sl])
        else:
            nc.vector.tensor_add(c_fold[:, sl], fc[:, sl], ig[:, sl])

    h_fold = pools["hfold"].tile([128, H], BF16, tag="hfold")
    hT = pools["hT"].tile([128, NK, BS], BF16, tag="hT")

    def tail(interleave_fn=None, prev_h=None):
        hpsA = pools["psum_hA"].tile([128, NSL * 2 + 1, BS], BF16, tag="hpsA")
        hpsB = pools["psum_hB"].tile([128, NSL * 2, BS], BF16, tag="hpsB")
        if prev_h is not None:
            # anti-HAM filler: the PE would otherwise idle ~1.4us between the
            # last gate matmul and the first h transpose, which trips the
            # clock-gate's idle window and halves the next cell's clock.
            # Transpose stale data (prev cell's h, always ready) into the
            # spare bank-A slot to keep the activity monitor fed.
            for _ in range(3):
                nc.tensor.transpose(hpsA[:, NSL * 2, :], prev_h[0:64, 0:128],
                                    idn2[0:64, :])
        for s in range(NSL):
            sl = slice(SW * s, SW * (s + 1))
            tc_t = gates.tile([128, SW], F32, tag="tcs")
            nc.scalar.activation(tc_t, c_fold[:, sl], AF.Tanh)
            o_t = gates.tile([128, SW], F32, tag="tcs")
            nc.scalar.activation(o_t, zp["o"][:, sl], AF.Sigmoid)
            nc.vector.tensor_mul(h_fold[:, sl], o_t, tc_t)
            # full-width transposes; each PSUM bank sees only one row-group
            for kk in (2 * s, 2 * s + 1):
                blk = slice(128 * kk, 128 * (kk + 1))
                nc.tensor.transpose(hpsA[:, kk, :], h_fold[0:64, blk], idn2[0:64, :])
                nc.tensor.transpose(hpsB[:, kk, :], h_fold[64:128, blk], idn2[64:128, :])
            nc.vector.tensor_copy(hT[:, 2 * s:2 * s + 2, :], hpsA[:, 2 * s:2 * s + 2, :])
            nc.vector.tensor_copy(hT[:, 4 + 2 * s:6 + 2 * s, :], hpsB[:, 2 * s:2 * s + 2, :])
            if interleave_fn is not None:
                interleave_fn(s)

    return hT, tail, h_fold


def _emit_pred_chain(nc, pools, wd_sb, hT):
    """Emit the 8 pred matmuls in hT production order; returns psum tile.
    Designed to be dropped into a cell tail via its interleave hook."""
    pd = pools["psum_p"].tile([128, BS], F32, tag="pd")

    def emit_slice(s):
        ks = [2 * s, 2 * s + 1, 4 + 2 * s, 5 + 2 * s]
        for j, k in enumerate(ks):
            nc.tensor.matmul(
                pd[:, :], wd_sb[:, k, :], hT[:, k, :],
                start=(s == 0 and j == 0), stop=(s == NSL - 1 and j == 3),
            )
    return pd, emit_slice


def _emit_pred_finish(nc, pools, pd, out_dram, step_idx, bd_sb=None):
    pdv = pools["temps"].tile([128, BS], F32, tag="pdv")
    if bd_sb is not None:
        nc.vector.tensor_scalar_add(pdv, pd[:, :], bd_sb[:, 0:1])
    else:
        nc.vector.tensor_copy(pdv[:, :], pd[:, :])
    nc.sync.dma_start(out=out_dram[step_idx], in_=pdv[:, :])
    predT = pools["predT"].tile([128, BS], BF16, tag="predT")
    nc.vector.tensor_copy(predT[:, :], pdv[:, :])
    return predT


def _build(has_b1, has_b2, has_bd):
    nc = bacc.Bacc("TRN2", target_bir_lowering=False, debug=False)

    XT = nc.declare_dram_parameter("xt", [F, T, BS], BF16, isOutput=False)
    U1 = nc.declare_dram_parameter("u1", [128, NK, G], BF16, isOutput=False)
    W1 = nc.declare_dram_parameter("w1", [F, G], BF16, isOutput=False)
    W2 = nc.declare_dram_parameter("w2", [128, NK, G], BF16, isOutput=False)
    WD = nc.declare_dram_parameter("wd", [128, NK, F], BF16, isOutput=False)
    IDN = nc.declare_dram_parameter("idn", [128, 64], BF16, isOutput=False)
    if has_b1:
        B1 = nc.declare_dram_parameter("b1f", [4, 128, H], F32, isOutput=False)
    if has_b2:
        B2 = nc.declare_dram_parameter("b2f", [4, 128, H], F32, isOutput=False)
    if has_bd:
        BD = nc.declare_dram_parameter("bdf", [128, 1], F32, isOutput=False)
    OUT = nc.declare_dram_parameter("out", [OUT_STEPS, F, BS], F32, isOutput=True)

    with tile.TileContext(nc) as tc, ExitStack() as ctx:
        consts = ctx.enter_context(tc.tile_pool(name="consts", bufs=1))
        pools = {
            "psum": ctx.enter_context(tc.tile_pool(name="psum", bufs=5, space="PSUM")),
            "psum_hA": ctx.enter_context(tc.tile_pool(name="psum_hA", bufs=1, space="PSUM")),
            "psum_hB": ctx.enter_context(tc.tile_pool(name="psum_hB", bufs=1, space="PSUM")),
            "psum_p": ctx.enter_context(tc.tile_pool(name="psum_p", bufs=1, space="PSUM")),
            "gates": ctx.enter_context(tc.tile_pool(name="gates", bufs=8)),
            "temps": ctx.enter_context(tc.tile_pool(name="temps", bufs=4)),
            "hfold": ctx.enter_context(tc.tile_pool(name="hfold", bufs=2)),
            "hT": ctx.enter_context(tc.tile_pool(name="hT", bufs=2)),
            "predT": ctx.enter_context(tc.tile_pool(name="predT", bufs=2)),
        }

        xt_sb = consts.tile([F, T, BS], BF16)
        u1_sb = consts.tile([128, NK, G], BF16)
        w1_sb = consts.tile([F, G], BF16)
        w2_sb = consts.tile([128, NK, G], BF16)
        wd_sb = consts.tile([128, NK, F], BF16)
        idn_sb = consts.tile([128, 64], BF16)
        nc.sync.dma_start(out=xt_sb[:], in_=XT[:])
        nc.sync.dma_start(out=w1_sb[:], in_=W1[:])
        for k in range(NK):  # split big weight DMAs so early steps start sooner
            nc.sync.dma_start(out=u1_sb[:, k, :], in_=U1[:, k, :])
        for k in range(NK):
            nc.sync.dma_start(out=w2_sb[:, k, :], in_=W2[:, k, :])
        nc.sync.dma_start(out=wd_sb[:], in_=WD[:])
        nc.sync.dma_start(out=idn_sb[:], in_=IDN[:])

        b1_tiles = b2_tiles = None
        if has_b1:
            b1_sb = consts.tile([4, 128, H], F32)
            nc.sync.dma_start(out=b1_sb[:], in_=B1[:])
            b1_tiles = {g: b1_sb[i] for i, g in enumerate(("i", "f", "g", "o"))}
        if has_b2:
            b2_sb = consts.tile([4, 128, H], F32)
            nc.sync.dma_start(out=b2_sb[:], in_=B2[:])
            b2_tiles = {g: b2_sb[i] for i, g in enumerate(("i", "f", "g", "o"))}
        bd_sb = None
        if has_bd:
            bd_sb = consts.tile([128, 1], F32)
            nc.sync.dma_start(out=bd_sb[:], in_=BD[:])

        c_fold = consts.tile([128, H], F32)  # persistent cell state

        def w1_rhs(off):
            return w1_sb[:, off:off + H]

        def u1_rhs(k):
            return lambda off: u1_sb[:, k, off:off + H]

        def w2_rhs(k):
            return lambda off: w2_sb[:, k, off:off + H]

        # ---- warmup: 64 steps of cell1 over the input sequence ----
        hT = None
        tail = None
        prev_h = None
        for t in range(T):
            x_term = (xt_sb[:, t, :], w1_rhs)
            if t == 0:
                zp = _emit_chains(nc, pools, [x_term], final=True)
                hT, tail, prev_h = _emit_cell(nc, pools, zp, c_fold, True, idn_sb, b1_tiles)
                continue
            # pre-start next cell's x-side chains so the PE has work during
            # the previous cell's tail, then emit that tail
            zp = _emit_chains(nc, pools, [x_term], final=False)
            tail()
            h_terms = [(hT[:, k, :], u1_rhs(k)) for k in K_PROD_ORDER]
            zp = _emit_chains(nc, pools, h_terms, zp=zp, final=True)
            hT, tail, prev_h = _emit_cell(nc, pools, zp, c_fold, False, idn_sb, b1_tiles)
        # ---- pred0 interleaved into the last warmup tail ----
        pd, pred_slice = _emit_pred_chain(nc, pools, wd_sb, hT)
        tail(pred_slice)
        predT = _emit_pred_finish(nc, pools, pd, OUT, 0, bd_sb)

        # ---- autoregressive: 31 steps of cell1 + cell2 ----
        for t in range(OUT_STEPS - 1):
            # predT is the LAST term so its producer chain (in the previous
            # tail) has slack while the h-terms stream
            terms1 = [(hT[:, k, :], u1_rhs(k)) for k in K_PROD_ORDER] + [(predT, w1_rhs)]
            zp = _emit_chains(nc, pools, terms1, final=True)
            h1T, tail1, h1_fold = _emit_cell(nc, pools, zp, c_fold, False, idn_sb, b1_tiles)
            tail1(None, prev_h)
            terms2 = [(h1T[:, k, :], w2_rhs(k)) for k in K_PROD_ORDER]
            zp = _emit_chains(nc, pools, terms2, final=True)
            hT, tail2, prev_h = _emit_cell(nc, pools, zp, c_fold, False, idn_sb, b2_tiles)
            pd, pred_slice = _emit_pred_chain(nc, pools, wd_sb, hT)
            tail2(pred_slice, h1_fold)
            predT = _emit_pred_finish(nc, pools, pd, OUT, t + 1, bd_sb)

    nc.compile()
    return nc


def _fold_bias(b):
    """[4096] gate bias -> [4, 128, 512] folded tiles in (i,f,g,o) order."""
    out = np.zeros((4, 128, H), np.float32)
    for gi, gname in enumerate(("i", "f", "g", "o")):
        off = GATE_OFF[gname]
        out[gi, 0:64, :] = b[off:off + H][None, :]
        out[gi, 64:128, :] = b[off + H:off + 2 * H][None, :]
    return out


def kernel(inputs, mean, var, W1, U1, b1, W2, U2, b2, Wd, bd):
    x = np.asarray(inputs, np.float32)
    mean = np.asarray(mean, np.float32)
    var = np.asarray(var, np.float32)
    inv = 1.0 / np.sqrt(var + EPS)
    xn = ((x - mean) * inv - mean) * inv  # reference normalizes twice

    W1 = np.asarray(W1, np.float32)
    U1k = np.asarray(U1, np.float32).reshape(NK, 128, G).transpose(1, 0, 2)
    W2U2 = (np.asarray(W2, np.float32) + np.asarray(U2, np.float32))
    W2k = W2U2.reshape(NK, 128, G).transpose(1, 0, 2)
    WDk = np.asarray(Wd, np.float32).reshape(NK, 128, F).transpose(1, 0, 2)
    idn2 = np.zeros((128, 64), np.float32)
    idn2[0:64] = np.eye(64)
    idn2[64:128] = np.eye(64)

    b1 = np.asarray(b1, np.float32)
    b2 = np.asarray(b2, np.float32)
    bd = np.asarray(bd, np.float32)
    has_b1 = bool(np.any(b1))
    has_b2 = bool(np.any(b2))
    has_bd = bool(np.any(bd))

    key = (has_b1, has_b2, has_bd)
    if key not in _BUILD_CACHE:
        _BUILD_CACHE[key] = _build(*key)
    nc = _BUILD_CACHE[key]

    bf = ml_dtypes.bfloat16
    shared = {
        "u1": np.ascontiguousarray(U1k).astype(bf),
        "w1": W1.astype(bf),
        "w2": np.ascontiguousarray(W2k).astype(bf),
        "wd": np.ascontiguousarray(WDk).astype(bf),
        "idn": idn2.astype(bf),
    }
    if has_b1:
        shared["b1f"] = _fold_bias(b1)
    if has_b2:
        shared["b2f"] = _fold_bias(b2)
    if has_bd:
        shared["bdf"] = bd.reshape(128, 1).astype(np.float32)

    in_maps = []
    for c in range(NCORES):
        shard = xn[c * BS:(c + 1) * BS]              # [64, 64, 128]
        xt = np.ascontiguousarray(shard.transpose(2, 1, 0)).astype(bf)
        m = dict(shared)
        m["xt"] = xt
        in_maps.append(m)

    res = run_bass_kernel_spmd(nc, in_maps, core_ids=list(range(NCORES)))
    kernel.last_results = res

    # per-core out: [32, 128 feat, 64 batch] -> [64, 32, 128]
    parts = [res.results[c]["out"].transpose(2, 0, 1) for c in range(NCORES)]
    return np.ascontiguousarray(np.concatenate(parts, axis=0), dtype=np.float32)
